# revision 42
# baseline (speedup 1.0000x reference)
"""Multi-head attention block (QKV proj + softmax attention + out proj) on 8 TRN2 cores.

Sharding: head-parallel. Each core c owns heads (2c, 2c+1) for both batch elements:
  - Wq/Wk/Wv column slice [:, c*128:(c+1)*128], Wo row slice [c*128:(c+1)*128, :]
  - Q.T/K.T in [head-dim, tok] layout for scores; V in DIRECT [tok, head-dim]
    layout (projected with lhsT=X.T so no PE transposes are needed for V).
  - scores in transposed layout S.T[k, q]; exp on ACT with the 1/sqrt(d) scale
    folded into the activation pre-scale.
  - attnV in the [q, d] layout: out[128q, 65] = P.T-slice.T @ [V | ones], so the
    PE contracts over all 128 k-partitions (full utilization: 65-cycle matmuls
    instead of 512-cycle ones) and the softmax denominator lands per-q-partition
    in column 64 -> normalization is a per-partition tensor_scalar multiply.
  - O[q, hc] normalized tiles are PE-transposed to O.T[hc, tok] for the
    out-projection, which emits a partial Y = O @ Wo_c; host sums the 8 partials.

Emission is a single software-pipelined stream over all (b, qgroup, kc) slots,
grouped into BLOCKS (4 or 8 kc each). A fill queue of future-block work (proj
groups, V tiles, epilogues, out-projections) is drained between slots; a unit
consumed by block B is always fully emitted before B's first instruction
(hardware constraint from A/B testing: same-block delivery of interleaved
units produces stale reads on silicon). scores for a block's first slot are
hoisted into the previous block when their deps are already emitted, so the
ACT engine never bubbles at block boundaries.

PSUM (8 banks): sp [128,1024]f32 x2 (4 banks) + op [128,2,4,128]f32 x1
(2 banks) + yp [128,512]f32 x2 (2 banks).
"""

import numpy as np
import ml_dtypes

B = 2
S = 2048
TOK = B * S
D = 1024
HD = 64
HC = 128  # head-cols per core: 2 heads x 64
NCORES = 8
KC = D // 128  # contraction chunks for the projections
NKT = S // 128  # k-token tiles per batch
SCALE = 0.125  # 1/sqrt(HD)
QG = 512  # query-group size per attention phase
NQG = S // QG  # phases per batch
NQT = QG // 128  # 128-query tiles per phase

_CACHE = {}


def _build_nc():
    import concourse.mybir as mybir
    import concourse.tile as tile
    from concourse import bacc
    from concourse.masks import make_identity

    f32 = mybir.dt.float32
    bf16 = mybir.dt.bfloat16
    Exp = mybir.ActivationFunctionType.Exp

    nc = bacc.Bacc("TRN2", target_bir_lowering=False, debug=False, num_devices=NCORES)
    xt_d = nc.dram_tensor("xt", [D, TOK], bf16, kind="ExternalInput")
    # weights host-packed as [128, KC*HC] (partition p row = concat over
    # chunks o of W[o*128+p, :]) so each load is 2KB-contiguous per partition
    wq_d = nc.dram_tensor("wq", [128, KC * HC], bf16, kind="ExternalInput")
    wk_d = nc.dram_tensor("wk", [128, KC * HC], bf16, kind="ExternalInput")
    wv_d = nc.dram_tensor("wv", [128, KC * HC], bf16, kind="ExternalInput")
    wo_d = nc.dram_tensor("wo", [HC, D], bf16, kind="ExternalInput")
    y_d = nc.dram_tensor("y", [TOK, D], bf16, kind="ExternalOutput")

    with tile.TileContext(nc) as tc:
        with (
            tc.tile_pool(name="consts", bufs=1) as consts,
            tc.tile_pool(name="persist", bufs=1) as persist,
            tc.tile_pool(name="xqp", bufs=4) as xqp,
            tc.tile_pool(name="ptp", bufs=3) as ptp,
            tc.tile_pool(name="miscp", bufs=2) as miscp,
            tc.tile_pool(name="ysbp", bufs=6) as ysbp,
            tc.tile_pool(name="aps", space="PSUM", bufs=1) as aps,
        ):
            # --- persistent SBUF ---
            w_sb = {}

            def load_weight(nm, d, eng):
                w = consts.tile([128, KC, HC], bf16, name=f"{nm}_sb", tag=nm)
                eng.dma_start(w[:], d.rearrange("p (o m) -> p o m", o=KC))
                w_sb[nm] = w
            load_weight("wk", wk_d, nc.sync)
            wo_sb = consts.tile([HC, D], bf16, name="wo_sb", tag="wo")
            ident = consts.tile([128, 128], bf16, name="ident", tag="ident")
            make_identity(nc, ident[:])

            qt = persist.tile([HC, TOK], bf16, name="qt", tag="qt")
            kt = persist.tile([HC, TOK], bf16, name="kt", tag="kt")
            # V direct layout per (batch, ktile, head): [tok-part, 65];
            # col 64 = ones (softmax denominator rides the PV matmul).
            vp = persist.tile([128, B, NKT, 2, 65], bf16, name="vp", tag="vp")
            ot = persist.tile([HC, TOK], bf16, name="ot", tag="ot")
            nc.gpsimd.memset(vp[:, :, :, :, 64:65], 1.0)

            xt_r = xt_d.rearrange("(o p) n -> p o n", p=128)
            xq_tiles = {}

            def get_xq(tq):
                if tq not in xq_tiles:
                    xq_tiles[tq] = xqp.tile([128, KC, 1024], bf16,
                                            name=f"xq{tq}", tag=f"xq{tq}", bufs=1)
                return xq_tiles[tq]

            def load_xq_half(tq, half, eng, pair_chunks=False):
                """Load 512 tokens (half a tq tile). pair_chunks splits into
                4 DMAs of 2 contraction-chunks so the first proj matmuls can
                chase the arriving data."""
                xq = get_xq(tq)
                c0 = tq * 1024 + half * 512
                step = 2 if pair_chunks else KC
                for kc in range(0, KC, step):
                    eng.dma_start(
                        xq[:, kc:kc + step, half * 512:(half + 1) * 512],
                        xt_r[:, kc:kc + step, c0:c0 + 512])

            def load_xq(tq, eng):
                xq = get_xq(tq)
                eng.dma_start(xq[:], xt_r[:, :, tq * 1024:(tq + 1) * 1024])

            # warm the ACT exp table off the critical path
            warm = miscp.tile([1, 64], f32, name="warm", tag="warm", bufs=1)
            nc.gpsimd.memset(warm[:], 0.0)
            nc.scalar.activation(warm[:], warm[:], Exp)

            proj_dst = {"q": qt, "k": kt}

            # units yield their approximate PE-ns cost per step (fill pacing)

            def proj_group(pname, g, copy_eng=None):
                """One [128,512] Q.T/K.T projection group; yields per matmul."""
                dst, w = proj_dst[pname], w_sb["w" + pname]
                tq, nch = g // 2, g % 2
                xq = get_xq(tq)
                ps = aps.tile([128, 512], f32, name=f"ps_{pname}{g}", tag="yp",
                              bufs=2)
                for kc in range(KC):
                    nc.tensor.matmul(ps[:], w[:, kc, :],
                                     xq[:, kc, nch * 512:(nch + 1) * 512],
                                     start=(kc == 0), stop=(kc == KC - 1))
                    yield 215
                (copy_eng or nc.vector).tensor_copy(
                    out=dst[:, g * 512:(g + 1) * 512], in_=ps[:])
                yield 10

            def vproj_unit(b, t):
                """V[tok-tile, 128hc] directly (lhsT = X.T slice): no transpose."""
                tq, off = (b * S + t * 128) // 1024, (b * S + t * 128) % 1024
                xq = get_xq(tq)
                ps = aps.tile([128, 512], f32, name=f"vps{b}{t}", tag="yp", bufs=2)
                for kc in range(KC):
                    nc.tensor.matmul(ps[:, 0:128], xq[:, kc, off:off + 128],
                                     w_sb["wv"][:, kc, :],
                                     start=(kc == 0), stop=(kc == KC - 1))
                    if kc % 2 == 1:
                        yield 110
                nc.vector.tensor_copy(
                    out=vp[:, b, t, :, 0:64],
                    in_=ps[:, 0:128].rearrange("p (h m) -> p h m", h=2))
                yield 10

            def outproj_unit(pi, tt, ysb_eng):
                """Y[tok-tile, :] for one 128-token tile."""
                b, q0, qg = phases[pi]
                t0 = b * S + q0 + tt * 128
                for odc in range(2):
                    yp = aps.tile([128, 512], f32, name="yp", tag="yp", bufs=2)
                    nc.tensor.matmul(yp[:], ot[:, t0:t0 + 128],
                                     wo_sb[:, odc * 512:(odc + 1) * 512],
                                     start=True, stop=True)
                    ysb = ysbp.tile([128, 512], bf16, name="ysb", tag="ysb")
                    ysb_eng.tensor_copy(out=ysb[:], in_=yp[:])
                    nc.sync.dma_start(
                        y_d[t0:t0 + 128, odc * 512:(odc + 1) * 512], ysb[:])
                    yield 225

            def epilogue_unit(pi, osb, rr):
                """Normalize O, transpose to ot, out-project. osb/rr were
                emitted inline at the end of the phase (kc==15)."""
                b, q0, qg = phases[pi]
                t0 = b * S + q0
                nqt = qg // 128
                otd = miscp.tile([128, NQT, 128], bf16, name="otd", tag="otd",
                                 bufs=2)
                for h in range(2):
                    for qt_i in range(nqt):
                        nc.vector.tensor_scalar_mul(
                            otd[:, qt_i, h * 64:(h + 1) * 64],
                            osb[:, h, qt_i, 0:64], rr[:, h, qt_i, :])
                    yield 10
                for qt_i in range(nqt):
                    tp = aps.tile([128, 128], bf16, name="tp", tag="yp", bufs=2)
                    nc.tensor.transpose(tp[:], otd[:, qt_i, :], ident[:])
                    nc.vector.tensor_copy(
                        out=ot[:, t0 + qt_i * 128:t0 + (qt_i + 1) * 128],
                        in_=tp[:])
                    yield 70

            # ---------------- fill machinery ----------------
            from collections import deque
            fillq = deque()  # of (key, generator)
            done_keys = set()
            cur = [None]

            def _step():
                """Advance the head unit one yield; returns PE-ns cost or
                None when the queue is dry."""
                while True:
                    if cur[0] is None:
                        if not fillq:
                            return None
                        cur[0] = fillq.popleft()
                    key, gen = cur[0]
                    c = next(gen, StopIteration)
                    if c is StopIteration:
                        done_keys.add(key)
                        cur[0] = None
                        continue
                    return c

            fill_carry = [0]  # overshoot debt carried into the next slot

            def fill(budget_ns):
                avail = budget_ns + fill_carry[0]
                while avail > 0:
                    c = _step()
                    if c is None:
                        fill_carry[0] = 0  # idle PE time is not bankable
                        return
                    avail -= c
                fill_carry[0] = max(avail, -600)

            def drain_until(*keys):
                while any(k not in done_keys for k in keys):
                    if _step() is None:
                        raise RuntimeError(f"fill queue dry, missing {keys}")

            def drain_all():
                while _step() is not None:
                    pass

            def push(key, gen, front=False):
                if front:
                    fillq.appendleft((key, gen))
                else:
                    fillq.append((key, gen))

            def promote(keys):
                """Move units with the given keys (in order) to the queue
                front so deadlines for the next block are met first."""
                want = [k for k in keys if k not in done_keys]
                if cur[0] is not None and cur[0][0] in want:
                    want.remove(cur[0][0])
                if not want:
                    return
                picked = {k: None for k in want}
                rest = deque()
                for key, gen in fillq:
                    if key in picked and picked[key] is None:
                        picked[key] = gen
                    else:
                        rest.append((key, gen))
                fillq.clear()
                for k in want:
                    if picked.get(k) is not None:
                        fillq.append((k, picked[k]))
                fillq.extend(rest)

            def run_now(key, gen):
                for _ in gen:
                    pass
                done_keys.add(key)

            # ---------------- attention stream ----------------
            # phases: (b, q0, qg_width). The final phase is only 128 queries
            # so the post-last-exp tail (normalize/transpose/outproj/DMA) is
            # short; the 384 phase absorbs the remainder.
            phases = [(0, 0, 512), (0, 512, 512), (0, 1024, 512),
                      (0, 1536, 512), (1, 0, 512), (1, 512, 512),
                      (1, 1024, 512), (1, 1536, 384), (1, 1920, 128)]
            NPH = len(phases)

            # blocks: (phase, kc0, kc1). Phase 0 starts with single-kc blocks
            # so the V tiles can stream in via fill (attnV lags one slot);
            # later phases split in half to relax fill deadlines.
            blocks = []
            for pi in range(NPH):
                bounds = (0, 1, 2, 3, 4, 8, 12, 16) if pi == 0 else (0, 8, 16)
                for i in range(len(bounds) - 1):
                    blocks.append((pi, bounds[i], bounds[i + 1]))

            def block_reqs(blk):
                """Emission deps a block's slots consume: its q/k groups, and
                (since attnV lags one slot) the V tiles for kc0-1..kc1-2 plus
                the previous phase's last V tile when the block opens a phase."""
                pi, kc0, kc1 = blk
                b, q0, qg = phases[pi]
                reqs = [("q", (b * S + q0) // 512),
                        ("q", (b * S + q0 + qg - 1) // 512)]
                for kc in range(kc0, kc1):
                    reqs.append(("k", (b * S + kc * 128) // 512))
                for kc in range(max(kc0 - 1, 0), kc1 - 1):
                    reqs.append(("v", (b, kc)))
                if kc0 == 0 and pi > 0:
                    reqs.append(("v", (phases[pi - 1][0], NKT - 1)))
                return reqs

            phase_state = {}  # pi -> dict(op=...)

            def scores(pi, kc):
                b, q0, qg = phases[pi]
                t0 = b * S + q0
                k0 = b * S + kc * 128
                sp = aps.tile([128, 2 * QG], f32, name="sp", tag="sp", bufs=2)
                for h in range(2):
                    nc.tensor.matmul(
                        sp[:, h * qg:(h + 1) * qg],
                        kt[h * 64:(h + 1) * 64, k0:k0 + 128],
                        qt[h * 64:(h + 1) * 64, t0:t0 + qg],
                        start=True, stop=True)
                return sp

            # global 1-slot software pipeline for scores across blocks
            pending_sp = [None]  # scores psum for the NEXT slot, if hoisted

            slots = [(pi, kc)
                     for (pi, kc0, kc1) in blocks for kc in range(kc0, kc1)]
            slot_block = {}
            for bi, (pi, kc0, kc1) in enumerate(blocks):
                for kc in range(kc0, kc1):
                    slot_block[(pi, kc)] = bi

            # head: ALL loads on the one sync queue, strict priority order —
            # anything on a second queue jumps ahead on the shared DMA
            # engines and delays the critical head loads. Criticals first
            # (wk already queued, tokens 0:512 of X.T chunk-paired, wq, wv),
            # then the bulk in deadline order.
            load_weight("wq", wq_d, nc.sync)
            load_xq_half(0, 0, nc.sync, pair_chunks=True)
            load_weight("wv", wv_d, nc.sync)
            load_xq_half(0, 1, nc.sync)
            load_xq_half(1, 0, nc.sync)
            load_xq_half(1, 1, nc.sync)
            nc.sync.dma_start(wo_sb[:], wo_d[:])
            load_xq(2, nc.sync)
            load_xq(3, nc.sync)

            # PE p-state warmup: back-to-back dummy transposes during the DMA
            # wait, so the first real matmuls run at full clock (the PE needs
            # ~3us of continuous busyness to ramp; any idle resets it)
            wps = aps.tile([128, 128], bf16, name="wps", tag="sp", bufs=2)
            for _ in range(40):
                nc.tensor.transpose(wps[:], ident[:], ident[:])

            # k(0) and q(0) lockstepped so both chase the arriving chunk
            # pairs; q(0)'s staging copy goes to Pool so it overlaps k(0)'s
            gk, gq = proj_group("k", 0), proj_group("q", 0, nc.gpsimd)
            for _ in range(KC + 1):
                next(gk, None)
                next(gq, None)
            done_keys.update([("k", 0), ("q", 0)])
            # first two V tiles before scores(0): the PE is otherwise idle
            # while the k/q staging copies land, and idling resets the p-state
            run_now(("v", (0, 0)), vproj_unit(0, 0))
            run_now(("v", (0, 1)), vproj_unit(0, 1))

            # fill queue: remaining production in rough need-order
            for t in range(2, 4):
                push(("v", (0, t)), vproj_unit(0, t))
            push(("k", 1), proj_group("k", 1))
            for t in range(4, 8):
                push(("v", (0, t)), vproj_unit(0, t))
            push(("k", 2), proj_group("k", 2))
            push(("k", 3), proj_group("k", 3))
            for t in range(8, 16):
                push(("v", (0, t)), vproj_unit(0, t))
            push(("q", 1), proj_group("q", 1))
            push(("q", 2), proj_group("q", 2))
            push(("q", 3), proj_group("q", 3))
            push(("k", 4), proj_group("k", 4))
            push(("k", 5), proj_group("k", 5))
            for t in range(8):
                push(("v", (1, t)), vproj_unit(1, t))
            push(("q", 4), proj_group("q", 4))
            push(("k", 6), proj_group("k", 6))
            push(("k", 7), proj_group("k", 7))
            for t in range(8, 16):
                push(("v", (1, t)), vproj_unit(1, t))
            push(("q", 5), proj_group("q", 5))
            push(("q", 6), proj_group("q", 6))
            push(("q", 7), proj_group("q", 7))

            def attnV(pi, kc, pt):
                b, q0, qg = phases[pi]
                nqt = qg // 128
                op = phase_state[pi]["op"]
                for h in range(2):
                    for qt_i in range(nqt):
                        nc.tensor.matmul(
                            op[:, h, qt_i, 0:65],
                            pt[:, h * qg + qt_i * 128:h * qg + (qt_i + 1) * 128],
                            vp[:, b, kc, h, 0:65],
                            start=(kc == 0), stop=(kc == NKT - 1))
                if kc == NKT - 1:
                    if pi == NPH - 1:
                        final_tail(pi, op)
                        del phase_state[pi]
                        return
                    # read psum out NOW (frees op for the next phase), split
                    # per (h, qt) region in attnV emission order so the next
                    # phase's attnV WARs pipeline region-by-region
                    osb = miscp.tile([128, 2, NQT, 65], f32, name="osb",
                                     tag="osb", bufs=2)
                    for h in range(2):
                        for qt_i in range(nqt):
                            nc.vector.tensor_copy(
                                out=osb[:, h, qt_i, :],
                                in_=op[:, h, qt_i, 0:65])
                    rr = miscp.tile([128, 2, NQT, 1], f32, name="rr", tag="rr",
                                    bufs=2)
                    nc.vector.reciprocal(rr[:, :, 0:nqt, :],
                                         osb[:, :, 0:nqt, 64:65])
                    for tt in range(nqt):
                        eng = nc.vector if tt % 2 == 0 else nc.gpsimd
                        push(("op", (pi, tt)), outproj_unit(pi, tt, eng))
                    push(("epi", pi), epilogue_unit(pi, osb, rr), front=True)
                    del phase_state[pi]

            def final_tail(pi, op):
                """Last (small) phase: pipelined normalize/transpose/outproj
                finish emitted inline after the last attnV."""
                b, q0, qg = phases[pi]
                t0 = b * S + q0
                nqt = qg // 128
                osb = miscp.tile([128, 2, NQT, 65], f32, name="osbF",
                                 tag="osb", bufs=2)
                rr = miscp.tile([128, 2, NQT, 1], f32, name="rrF", tag="rr",
                                bufs=2)
                otd = miscp.tile([128, NQT, 128], bf16, name="otdF", tag="otd",
                                 bufs=2)
                for qt_i in range(nqt):
                    nc.vector.tensor_copy(out=osb[:, :, qt_i, :],
                                          in_=op[:, :, qt_i, 0:65])
                    nc.vector.reciprocal(rr[:, :, qt_i, :],
                                         osb[:, :, qt_i, 64:65])
                    for h in range(2):
                        nc.vector.tensor_scalar_mul(
                            otd[:, qt_i, h * 64:(h + 1) * 64],
                            osb[:, h, qt_i, 0:64], rr[:, h, qt_i, :])
                    tp = aps.tile([128, 128], bf16, name="tpF", tag="yp",
                                  bufs=2)
                    nc.tensor.transpose(tp[:], otd[:, qt_i, :], ident[:])
                    nc.vector.tensor_copy(
                        out=ot[:, t0 + qt_i * 128:t0 + (qt_i + 1) * 128],
                        in_=tp[:])
                    tq0 = t0 + qt_i * 128
                    for odc in range(2):
                        yp = aps.tile([128, 512], f32, name="ypF", tag="yp",
                                      bufs=2)
                        nc.tensor.matmul(yp[:], ot[:, tq0:tq0 + 128],
                                         wo_sb[:, odc * 512:(odc + 1) * 512],
                                         start=True, stop=True)
                        ysb = ysbp.tile([128, 512], bf16, name="ysbF",
                                        tag="ysb")
                        eng = nc.vector if odc == 0 else nc.gpsimd
                        eng.tensor_copy(out=ysb[:], in_=yp[:])
                        nc.sync.dma_start(
                            y_d[tq0:tq0 + 128, odc * 512:(odc + 1) * 512],
                            ysb[:])

            def sc_reqs(npi, nkc):
                nb, nq0, nqg = phases[npi]
                return (("q", (nb * S + nq0) // 512),
                        ("q", (nb * S + nq0 + nqg - 1) // 512),
                        ("k", (nb * S + nkc * 128) // 512))

            lagged = [None]  # (pi, kc, pt) whose attnV is one slot behind

            for si, (pi, kc) in enumerate(slots):
                b, q0, qg = phases[pi]
                bi = slot_block[(pi, kc)]
                blk = blocks[bi]
                if kc == blk[1]:  # block start
                    drain_until(*block_reqs(blk))
                    if bi + 1 < len(blocks):
                        promote(block_reqs(blocks[bi + 1]))
                if kc == 0:
                    phase_state[pi] = {
                        "op": aps.tile([128, 2, NQT, 128], f32,
                                       name=f"op{pi}", tag="op", bufs=1)}
                # scores for this slot: hoisted already, or emit now
                if pending_sp[0] is not None:
                    sp_cur = pending_sp[0]
                    pending_sp[0] = None
                else:
                    sp_cur = scores(pi, kc)

                # hoist scores for the next slot if its own deps (its q-group
                # and k-group, not the whole block's) are emitted
                nxt = slots[si + 1] if si + 1 < len(slots) else None
                if nxt is not None:
                    if slot_block[nxt] == bi or all(
                            k in done_keys for k in sc_reqs(*nxt)):
                        pending_sp[0] = scores(*nxt)

                pt = ptp.tile([128, 2 * QG], bf16, name="pt", tag="pt", bufs=3)
                nc.scalar.activation(pt[:, 0:2 * qg], sp_cur[:, 0:2 * qg],
                                     Exp, scale=SCALE)

                # fill runs in the window where attnV would otherwise make
                # the in-order PE queue wait on exp: budget = ACT slot time
                # minus this slot's own PE pipeline work minus margin
                act_ns = int(2 * qg * 0.833) + 185
                pipe_ns = 2 * int(qg * 0.42) + (qg // 128) * 54
                fill(act_ns - pipe_ns)

                # second chance: deps may have completed during this slot's
                # fill (cross-block hoist, boundary slots only)
                if nxt is not None and pending_sp[0] is None and all(
                        k in done_keys for k in sc_reqs(*nxt)):
                    pending_sp[0] = scores(*nxt)

                # attnV one slot behind: its exp finished during the previous
                # slot, so the PE never stalls on it
                if lagged[0] is not None:
                    attnV(*lagged[0])
                lagged[0] = (pi, kc, pt)
            # flush leftovers first so nothing is emitted after (and thus
            # executes after) the final per-qt tail
            drain_all()
            attnV(*lagged[0])
    nc.compile()
    return nc


def get_nc():
    if "nc" not in _CACHE:
        _CACHE["nc"] = _build_nc()
    return _CACHE["nc"]


def _pack_w(w_slice):
    """[D, HC] -> [128, KC*HC]: partition p row = concat_o W[o*128+p, :]."""
    bf = ml_dtypes.bfloat16
    return np.ascontiguousarray(
        w_slice.reshape(KC, 128, HC).transpose(1, 0, 2).reshape(128, KC * HC)
    ).astype(bf)


def make_in_maps(hidden_states, Wq, Wk, Wv, Wo):
    bf = ml_dtypes.bfloat16
    X = np.ascontiguousarray(np.asarray(hidden_states, np.float32).reshape(TOK, D))
    xt = np.ascontiguousarray(X.T).astype(bf)
    Wq = np.asarray(Wq, np.float32)
    Wk = np.asarray(Wk, np.float32)
    Wv = np.asarray(Wv, np.float32)
    Wo = np.asarray(Wo, np.float32)
    in_maps = []
    for c in range(NCORES):
        sl = slice(c * HC, (c + 1) * HC)
        in_maps.append({
            "xt": xt,
            "wq": _pack_w(Wq[:, sl]),
            "wk": _pack_w(Wk[:, sl]),
            "wv": _pack_w(Wv[:, sl]),
            "wo": np.ascontiguousarray(Wo[sl, :]).astype(bf),
        })
    return in_maps


def kernel(hidden_states, Wq, Wk, Wv, Wo, bo):
    from concourse.bass_utils import run_bass_kernel_spmd

    nc = get_nc()
    in_maps = make_in_maps(hidden_states, Wq, Wk, Wv, Wo)
    res = run_bass_kernel_spmd(nc, in_maps, list(range(NCORES)))
    _CACHE["last_result"] = res
    y = np.zeros((TOK, D), np.float32)
    for c in range(NCORES):
        y += np.asarray(res.results[c]["y"], np.float32)
    out = y.reshape(B, S, D) + np.asarray(bo, np.float32)[None, None, :]
    return out.astype(np.float32)


# revision 43
# speedup vs baseline: 1.0082x; 1.0082x over previous
"""Multi-head attention block (QKV proj + softmax attention + out proj) on 8 TRN2 cores.

Sharding: head-parallel. Each core c owns heads (2c, 2c+1) for both batch elements:
  - Wq/Wk/Wv column slice [:, c*128:(c+1)*128], Wo row slice [c*128:(c+1)*128, :]
  - Q.T/K.T in [head-dim, tok] layout for scores; V in DIRECT [tok, head-dim]
    layout (projected with lhsT=X.T so no PE transposes are needed for V).
  - scores in transposed layout S.T[k, q]; exp on ACT with the 1/sqrt(d) scale
    folded into the activation pre-scale.
  - attnV in the [q, d] layout: out[128q, 65] = P.T-slice.T @ [V | ones], so the
    PE contracts over all 128 k-partitions (full utilization: 65-cycle matmuls
    instead of 512-cycle ones) and the softmax denominator lands per-q-partition
    in column 64 -> normalization is a per-partition tensor_scalar multiply.
  - O[q, hc] normalized tiles are PE-transposed to O.T[hc, tok] for the
    out-projection, which emits a partial Y = O @ Wo_c; host sums the 8 partials.

Emission is a single software-pipelined stream over all (b, qgroup, kc) slots,
grouped into BLOCKS (4 or 8 kc each). A fill queue of future-block work (proj
groups, V tiles, epilogues, out-projections) is drained between slots; a unit
consumed by block B is always fully emitted before B's first instruction
(hardware constraint from A/B testing: same-block delivery of interleaved
units produces stale reads on silicon). scores for a block's first slot are
hoisted into the previous block when their deps are already emitted, so the
ACT engine never bubbles at block boundaries.

PSUM (8 banks): sp [128,1024]f32 x2 (4 banks) + op [128,2,4,128]f32 x1
(2 banks) + yp [128,512]f32 x2 (2 banks).
"""

import numpy as np
import ml_dtypes

B = 2
S = 2048
TOK = B * S
D = 1024
HD = 64
HC = 128  # head-cols per core: 2 heads x 64
NCORES = 8
KC = D // 128  # contraction chunks for the projections
NKT = S // 128  # k-token tiles per batch
SCALE = 0.125  # 1/sqrt(HD)
QG = 512  # query-group size per attention phase
NQG = S // QG  # phases per batch
NQT = QG // 128  # 128-query tiles per phase

_CACHE = {}


def _build_nc():
    import concourse.mybir as mybir
    import concourse.tile as tile
    from concourse import bacc
    from concourse.masks import make_identity

    f32 = mybir.dt.float32
    bf16 = mybir.dt.bfloat16
    Exp = mybir.ActivationFunctionType.Exp

    nc = bacc.Bacc("TRN2", target_bir_lowering=False, debug=False, num_devices=NCORES)
    xt_d = nc.dram_tensor("xt", [D, TOK], bf16, kind="ExternalInput")
    # weights host-packed as [128, KC*HC] (partition p row = concat over
    # chunks o of W[o*128+p, :]) so each load is 2KB-contiguous per partition
    wq_d = nc.dram_tensor("wq", [128, KC * HC], bf16, kind="ExternalInput")
    wk_d = nc.dram_tensor("wk", [128, KC * HC], bf16, kind="ExternalInput")
    wv_d = nc.dram_tensor("wv", [128, KC * HC], bf16, kind="ExternalInput")
    wo_d = nc.dram_tensor("wo", [HC, D], bf16, kind="ExternalInput")
    y_d = nc.dram_tensor("y", [TOK, D], bf16, kind="ExternalOutput")

    with tile.TileContext(nc) as tc:
        with (
            tc.tile_pool(name="consts", bufs=1) as consts,
            tc.tile_pool(name="persist", bufs=1) as persist,
            tc.tile_pool(name="xqp", bufs=4) as xqp,
            tc.tile_pool(name="ptp", bufs=3) as ptp,
            tc.tile_pool(name="miscp", bufs=2) as miscp,
            tc.tile_pool(name="ysbp", bufs=6) as ysbp,
            tc.tile_pool(name="aps", space="PSUM", bufs=1) as aps,
        ):
            # --- persistent SBUF ---
            w_sb = {}

            def load_weight(nm, d, eng):
                w = consts.tile([128, KC, HC], bf16, name=f"{nm}_sb", tag=nm)
                eng.dma_start(w[:], d.rearrange("p (o m) -> p o m", o=KC))
                w_sb[nm] = w
            load_weight("wk", wk_d, nc.sync)
            wo_sb = consts.tile([HC, D], bf16, name="wo_sb", tag="wo")
            ident = consts.tile([128, 128], bf16, name="ident", tag="ident")
            make_identity(nc, ident[:])

            qt = persist.tile([HC, TOK], bf16, name="qt", tag="qt")
            kt = persist.tile([HC, TOK], bf16, name="kt", tag="kt")
            # V direct layout per (batch, ktile, head): [tok-part, 65];
            # col 64 = ones (softmax denominator rides the PV matmul).
            vp = persist.tile([128, B, NKT, 2, 65], bf16, name="vp", tag="vp")
            ot = persist.tile([HC, TOK], bf16, name="ot", tag="ot")
            nc.gpsimd.memset(vp[:, :, :, :, 64:65], 1.0)

            xt_r = xt_d.rearrange("(o p) n -> p o n", p=128)
            xq_tiles = {}

            def get_xq(tq):
                if tq not in xq_tiles:
                    xq_tiles[tq] = xqp.tile([128, KC, 1024], bf16,
                                            name=f"xq{tq}", tag=f"xq{tq}", bufs=1)
                return xq_tiles[tq]

            def load_xq_half(tq, half, eng, pair_chunks=False):
                """Load 512 tokens (half a tq tile). pair_chunks splits into
                4 DMAs of 2 contraction-chunks so the first proj matmuls can
                chase the arriving data."""
                xq = get_xq(tq)
                c0 = tq * 1024 + half * 512
                step = 2 if pair_chunks else KC
                for kc in range(0, KC, step):
                    eng.dma_start(
                        xq[:, kc:kc + step, half * 512:(half + 1) * 512],
                        xt_r[:, kc:kc + step, c0:c0 + 512])

            def load_xq(tq, eng):
                xq = get_xq(tq)
                eng.dma_start(xq[:], xt_r[:, :, tq * 1024:(tq + 1) * 1024])

            # warm the ACT exp table off the critical path
            warm = miscp.tile([1, 64], f32, name="warm", tag="warm", bufs=1)
            nc.gpsimd.memset(warm[:], 0.0)
            nc.scalar.activation(warm[:], warm[:], Exp)

            proj_dst = {"q": qt, "k": kt}

            # units yield their approximate PE-ns cost per step (fill pacing)

            def proj_group(pname, g, copy_eng=None):
                """One [128,512] Q.T/K.T projection group; yields per matmul."""
                dst, w = proj_dst[pname], w_sb["w" + pname]
                tq, nch = g // 2, g % 2
                xq = get_xq(tq)
                ps = aps.tile([128, 512], f32, name=f"ps_{pname}{g}", tag="yp",
                              bufs=2)
                for kc in range(KC):
                    nc.tensor.matmul(ps[:], w[:, kc, :],
                                     xq[:, kc, nch * 512:(nch + 1) * 512],
                                     start=(kc == 0), stop=(kc == KC - 1))
                    yield 215
                (copy_eng or nc.vector).tensor_copy(
                    out=dst[:, g * 512:(g + 1) * 512], in_=ps[:])
                yield 10

            def vproj_unit(b, t):
                """V[tok-tile, 128hc] directly (lhsT = X.T slice): no transpose."""
                tq, off = (b * S + t * 128) // 1024, (b * S + t * 128) % 1024
                xq = get_xq(tq)
                ps = aps.tile([128, 512], f32, name=f"vps{b}{t}", tag="yp", bufs=2)
                for kc in range(KC):
                    nc.tensor.matmul(ps[:, 0:128], xq[:, kc, off:off + 128],
                                     w_sb["wv"][:, kc, :],
                                     start=(kc == 0), stop=(kc == KC - 1))
                    if kc % 2 == 1:
                        yield 110
                nc.vector.tensor_copy(
                    out=vp[:, b, t, :, 0:64],
                    in_=ps[:, 0:128].rearrange("p (h m) -> p h m", h=2))
                yield 10

            def outproj_unit(pi, tt, ysb_eng):
                """Y[tok-tile, :] for one 128-token tile."""
                b, q0, qg = phases[pi]
                t0 = b * S + q0 + tt * 128
                for odc in range(2):
                    yp = aps.tile([128, 512], f32, name="yp", tag="yp", bufs=2)
                    nc.tensor.matmul(yp[:], ot[:, t0:t0 + 128],
                                     wo_sb[:, odc * 512:(odc + 1) * 512],
                                     start=True, stop=True)
                    ysb = ysbp.tile([128, 512], bf16, name="ysb", tag="ysb")
                    ysb_eng.tensor_copy(out=ysb[:], in_=yp[:])
                    nc.sync.dma_start(
                        y_d[t0:t0 + 128, odc * 512:(odc + 1) * 512], ysb[:])
                    yield 225

            def epilogue_unit(pi, osb, rr):
                """Normalize O, transpose to ot, out-project. osb/rr were
                emitted inline at the end of the phase (kc==15)."""
                b, q0, qg = phases[pi]
                t0 = b * S + q0
                nqt = qg // 128
                otd = miscp.tile([128, NQT, 128], bf16, name="otd", tag="otd",
                                 bufs=2)
                for h in range(2):
                    for qt_i in range(nqt):
                        nc.vector.tensor_scalar_mul(
                            otd[:, qt_i, h * 64:(h + 1) * 64],
                            osb[:, h, qt_i, 0:64], rr[:, h, qt_i, :])
                    yield 10
                for qt_i in range(nqt):
                    tp = aps.tile([128, 128], bf16, name="tp", tag="yp", bufs=2)
                    nc.tensor.transpose(tp[:], otd[:, qt_i, :], ident[:])
                    nc.vector.tensor_copy(
                        out=ot[:, t0 + qt_i * 128:t0 + (qt_i + 1) * 128],
                        in_=tp[:])
                    yield 70

            # ---------------- fill machinery ----------------
            from collections import deque
            fillq = deque()  # of (key, generator)
            done_keys = set()
            cur = [None]

            def _step():
                """Advance the head unit one yield; returns PE-ns cost or
                None when the queue is dry."""
                while True:
                    if cur[0] is None:
                        if not fillq:
                            return None
                        cur[0] = fillq.popleft()
                    key, gen = cur[0]
                    c = next(gen, StopIteration)
                    if c is StopIteration:
                        done_keys.add(key)
                        cur[0] = None
                        continue
                    return c

            fill_carry = [0]  # overshoot debt carried into the next slot

            def fill(budget_ns):
                avail = budget_ns + fill_carry[0]
                while avail > 0:
                    c = _step()
                    if c is None:
                        fill_carry[0] = 0  # idle PE time is not bankable
                        return
                    avail -= c
                fill_carry[0] = max(avail, -600)

            def drain_until(*keys):
                while any(k not in done_keys for k in keys):
                    if _step() is None:
                        raise RuntimeError(f"fill queue dry, missing {keys}")

            def drain_all():
                while _step() is not None:
                    pass

            def push(key, gen, front=False):
                if front:
                    fillq.appendleft((key, gen))
                else:
                    fillq.append((key, gen))

            def promote(keys):
                """Move units with the given keys (in order) to the queue
                front so deadlines for the next block are met first."""
                want = [k for k in keys if k not in done_keys]
                if cur[0] is not None and cur[0][0] in want:
                    want.remove(cur[0][0])
                if not want:
                    return
                picked = {k: None for k in want}
                rest = deque()
                for key, gen in fillq:
                    if key in picked and picked[key] is None:
                        picked[key] = gen
                    else:
                        rest.append((key, gen))
                fillq.clear()
                for k in want:
                    if picked.get(k) is not None:
                        fillq.append((k, picked[k]))
                fillq.extend(rest)

            def run_now(key, gen):
                for _ in gen:
                    pass
                done_keys.add(key)

            # ---------------- attention stream ----------------
            # phases: (b, q0, qg_width). The final phase is only 128 queries
            # so the post-last-exp tail (normalize/transpose/outproj/DMA) is
            # short; the 384 phase absorbs the remainder.
            phases = [(0, 0, 512), (0, 512, 512), (0, 1024, 512),
                      (0, 1536, 512), (1, 0, 512), (1, 512, 512),
                      (1, 1024, 512), (1, 1536, 384), (1, 1920, 128)]
            NPH = len(phases)

            # blocks: (phase, kc0, kc1). Phase 0 starts with single-kc blocks
            # so the V tiles can stream in via fill (attnV lags one slot);
            # later phases split in half to relax fill deadlines.
            blocks = []
            for pi in range(NPH):
                bounds = (0, 1, 2, 3, 4, 8, 12, 16) if pi == 0 else (0, 8, 16)
                for i in range(len(bounds) - 1):
                    blocks.append((pi, bounds[i], bounds[i + 1]))

            def block_reqs(blk):
                """Emission deps a block's slots consume: its q/k groups, and
                (since attnV lags one slot) the V tiles for kc0-1..kc1-2 plus
                the previous phase's last V tile when the block opens a phase."""
                pi, kc0, kc1 = blk
                b, q0, qg = phases[pi]
                reqs = [("q", (b * S + q0) // 512),
                        ("q", (b * S + q0 + qg - 1) // 512)]
                for kc in range(kc0, kc1):
                    reqs.append(("k", (b * S + kc * 128) // 512))
                for kc in range(max(kc0 - 1, 0), kc1 - 1):
                    reqs.append(("v", (b, kc)))
                if kc0 == 0 and pi > 0:
                    reqs.append(("v", (phases[pi - 1][0], NKT - 1)))
                return reqs

            phase_state = {}  # pi -> dict(op=...)

            def scores(pi, kc):
                b, q0, qg = phases[pi]
                t0 = b * S + q0
                k0 = b * S + kc * 128
                sp = aps.tile([128, 2 * QG], f32, name="sp", tag="sp", bufs=2)
                for h in range(2):
                    nc.tensor.matmul(
                        sp[:, h * qg:(h + 1) * qg],
                        kt[h * 64:(h + 1) * 64, k0:k0 + 128],
                        qt[h * 64:(h + 1) * 64, t0:t0 + qg],
                        start=True, stop=True)
                return sp

            # global 1-slot software pipeline for scores across blocks
            pending_sp = [None]  # scores psum for the NEXT slot, if hoisted

            slots = [(pi, kc)
                     for (pi, kc0, kc1) in blocks for kc in range(kc0, kc1)]
            slot_block = {}
            for bi, (pi, kc0, kc1) in enumerate(blocks):
                for kc in range(kc0, kc1):
                    slot_block[(pi, kc)] = bi

            # head: ALL loads on the one sync queue, strict priority order —
            # anything on a second queue jumps ahead on the shared DMA
            # engines and delays the critical head loads. Criticals first
            # (wk already queued, tokens 0:512 of X.T chunk-paired, wq, wv),
            # then the bulk in deadline order.
            load_weight("wq", wq_d, nc.sync)
            load_xq_half(0, 0, nc.sync, pair_chunks=True)
            load_weight("wv", wv_d, nc.sync)
            load_xq_half(0, 1, nc.sync)
            load_xq_half(1, 0, nc.sync)
            load_xq_half(1, 1, nc.sync)
            nc.sync.dma_start(wo_sb[:], wo_d[:])
            load_xq(2, nc.sync)
            load_xq(3, nc.sync)

            # PE p-state warmup: back-to-back dummy transposes during the DMA
            # wait, so the first real matmuls run at full clock (the PE needs
            # ~3us of continuous busyness to ramp; any idle resets it)
            wps = aps.tile([128, 128], bf16, name="wps", tag="sp", bufs=2)
            for _ in range(40):
                nc.tensor.transpose(wps[:], ident[:], ident[:])

            # k(0) and q(0) lockstepped so both chase the arriving chunk
            # pairs; q(0)'s staging copy goes to Pool so it overlaps k(0)'s
            gk, gq = proj_group("k", 0), proj_group("q", 0, nc.gpsimd)
            for _ in range(KC + 1):
                next(gk, None)
                next(gq, None)
            done_keys.update([("k", 0), ("q", 0)])
            # first two V tiles before scores(0): the PE is otherwise idle
            # while the k/q staging copies land, and idling resets the p-state
            run_now(("v", (0, 0)), vproj_unit(0, 0))
            run_now(("v", (0, 1)), vproj_unit(0, 1))

            # fill queue: remaining production in rough need-order
            for t in range(2, 4):
                push(("v", (0, t)), vproj_unit(0, t))
            push(("k", 1), proj_group("k", 1))
            for t in range(4, 8):
                push(("v", (0, t)), vproj_unit(0, t))
            push(("k", 2), proj_group("k", 2))
            push(("k", 3), proj_group("k", 3))
            for t in range(8, 16):
                push(("v", (0, t)), vproj_unit(0, t))
            push(("q", 1), proj_group("q", 1))
            push(("q", 2), proj_group("q", 2))
            push(("q", 3), proj_group("q", 3))
            push(("k", 4), proj_group("k", 4))
            push(("k", 5), proj_group("k", 5))
            for t in range(8):
                push(("v", (1, t)), vproj_unit(1, t))
            push(("q", 4), proj_group("q", 4))
            push(("k", 6), proj_group("k", 6))
            push(("k", 7), proj_group("k", 7))
            for t in range(8, 16):
                push(("v", (1, t)), vproj_unit(1, t))
            push(("q", 5), proj_group("q", 5))
            push(("q", 6), proj_group("q", 6))
            push(("q", 7), proj_group("q", 7))

            def attnV(pi, kc, pt):
                b, q0, qg = phases[pi]
                nqt = qg // 128
                op = phase_state[pi]["op"]
                for h in range(2):
                    for qt_i in range(nqt):
                        nc.tensor.matmul(
                            op[:, h, qt_i, 0:65],
                            pt[:, h * qg + qt_i * 128:h * qg + (qt_i + 1) * 128],
                            vp[:, b, kc, h, 0:65],
                            start=(kc == 0), stop=(kc == NKT - 1))
                if kc == NKT - 1:
                    if pi == NPH - 1:
                        final_tail(pi, op)
                        del phase_state[pi]
                        return
                    # read psum out NOW (frees op for the next phase), split
                    # per (h, qt) region in attnV emission order so the next
                    # phase's attnV WARs pipeline region-by-region
                    osb = miscp.tile([128, 2, NQT, 65], f32, name="osb",
                                     tag="osb", bufs=2)
                    for h in range(2):
                        for qt_i in range(nqt):
                            nc.vector.tensor_copy(
                                out=osb[:, h, qt_i, :],
                                in_=op[:, h, qt_i, 0:65])
                    rr = miscp.tile([128, 2, NQT, 1], f32, name="rr", tag="rr",
                                    bufs=2)
                    nc.vector.reciprocal(rr[:, :, 0:nqt, :],
                                         osb[:, :, 0:nqt, 64:65])
                    for tt in range(nqt):
                        eng = nc.vector if tt % 2 == 0 else nc.gpsimd
                        push(("op", (pi, tt)), outproj_unit(pi, tt, eng))
                    push(("epi", pi), epilogue_unit(pi, osb, rr), front=True)
                    del phase_state[pi]

            def final_tail(pi, op):
                """Last (small) phase: pipelined normalize/transpose/outproj
                finish emitted inline after the last attnV."""
                b, q0, qg = phases[pi]
                t0 = b * S + q0
                nqt = qg // 128
                osb = miscp.tile([128, 2, NQT, 65], f32, name="osbF",
                                 tag="osb", bufs=2)
                rr = miscp.tile([128, 2, NQT, 1], f32, name="rrF", tag="rr",
                                bufs=2)
                otd = miscp.tile([128, NQT, 128], bf16, name="otdF", tag="otd",
                                 bufs=2)
                for qt_i in range(nqt):
                    nc.vector.tensor_copy(out=osb[:, :, qt_i, :],
                                          in_=op[:, :, qt_i, 0:65])
                    nc.vector.reciprocal(rr[:, :, qt_i, :],
                                         osb[:, :, qt_i, 64:65])
                    for h in range(2):
                        nc.vector.tensor_scalar_mul(
                            otd[:, qt_i, h * 64:(h + 1) * 64],
                            osb[:, h, qt_i, 0:64], rr[:, h, qt_i, :])
                    tp = aps.tile([128, 128], bf16, name="tpF", tag="yp",
                                  bufs=2)
                    nc.tensor.transpose(tp[:], otd[:, qt_i, :], ident[:])
                    nc.vector.tensor_copy(
                        out=ot[:, t0 + qt_i * 128:t0 + (qt_i + 1) * 128],
                        in_=tp[:])
                    tq0 = t0 + qt_i * 128
                    for odc in range(2):
                        yp = aps.tile([128, 512], f32, name="ypF", tag="yp",
                                      bufs=2)
                        nc.tensor.matmul(yp[:], ot[:, tq0:tq0 + 128],
                                         wo_sb[:, odc * 512:(odc + 1) * 512],
                                         start=True, stop=True)
                        ysb = ysbp.tile([128, 512], bf16, name="ysbF",
                                        tag="ysb")
                        eng = nc.vector if odc == 0 else nc.gpsimd
                        eng.tensor_copy(out=ysb[:], in_=yp[:])
                        nc.sync.dma_start(
                            y_d[tq0:tq0 + 128, odc * 512:(odc + 1) * 512],
                            ysb[:])

            def sc_reqs(npi, nkc):
                nb, nq0, nqg = phases[npi]
                return (("q", (nb * S + nq0) // 512),
                        ("q", (nb * S + nq0 + nqg - 1) // 512),
                        ("k", (nb * S + nkc * 128) // 512))

            lagged = [None]  # (pi, kc, pt) whose attnV is one slot behind

            for si, (pi, kc) in enumerate(slots):
                b, q0, qg = phases[pi]
                bi = slot_block[(pi, kc)]
                blk = blocks[bi]
                if kc == blk[1]:  # block start
                    drain_until(*block_reqs(blk))
                    if bi + 1 < len(blocks):
                        promote(block_reqs(blocks[bi + 1]))
                if kc == 0:
                    phase_state[pi] = {
                        "op": aps.tile([128, 2, NQT, 128], f32,
                                       name=f"op{pi}", tag="op", bufs=1)}
                # scores for this slot: hoisted already, or emit now
                if pending_sp[0] is not None:
                    sp_cur = pending_sp[0]
                    pending_sp[0] = None
                else:
                    sp_cur = scores(pi, kc)

                # hoist scores for the next slot if its own deps (its q-group
                # and k-group, not the whole block's) are emitted
                nxt = slots[si + 1] if si + 1 < len(slots) else None
                if nxt is not None:
                    if slot_block[nxt] == bi or all(
                            k in done_keys for k in sc_reqs(*nxt)):
                        pending_sp[0] = scores(*nxt)

                pt = ptp.tile([128, 2 * QG], bf16, name="pt", tag="pt", bufs=3)
                nc.scalar.activation(pt[:, 0:2 * qg], sp_cur[:, 0:2 * qg],
                                     Exp, scale=SCALE)

                # fill runs in the window where attnV would otherwise make
                # the in-order PE queue wait on exp: budget = ACT slot time
                # minus this slot's own PE pipeline work minus margin
                act_ns = int(2 * qg * 0.833) + 185
                pipe_ns = 2 * int(qg * 0.42) + (qg // 128) * 54
                fill(act_ns - pipe_ns - 40)

                # second chance: deps may have completed during this slot's
                # fill (cross-block hoist, boundary slots only)
                if nxt is not None and pending_sp[0] is None and all(
                        k in done_keys for k in sc_reqs(*nxt)):
                    pending_sp[0] = scores(*nxt)

                # attnV one slot behind: its exp finished during the previous
                # slot, so the PE never stalls on it
                if lagged[0] is not None:
                    attnV(*lagged[0])
                lagged[0] = (pi, kc, pt)
            # flush leftovers first so nothing is emitted after (and thus
            # executes after) the final per-qt tail
            drain_all()
            attnV(*lagged[0])
    nc.compile()
    return nc


def get_nc():
    if "nc" not in _CACHE:
        _CACHE["nc"] = _build_nc()
    return _CACHE["nc"]


def _pack_w(w_slice):
    """[D, HC] -> [128, KC*HC]: partition p row = concat_o W[o*128+p, :]."""
    bf = ml_dtypes.bfloat16
    return np.ascontiguousarray(
        w_slice.reshape(KC, 128, HC).transpose(1, 0, 2).reshape(128, KC * HC)
    ).astype(bf)


def make_in_maps(hidden_states, Wq, Wk, Wv, Wo):
    bf = ml_dtypes.bfloat16
    X = np.ascontiguousarray(np.asarray(hidden_states, np.float32).reshape(TOK, D))
    xt = np.ascontiguousarray(X.T).astype(bf)
    Wq = np.asarray(Wq, np.float32)
    Wk = np.asarray(Wk, np.float32)
    Wv = np.asarray(Wv, np.float32)
    Wo = np.asarray(Wo, np.float32)
    in_maps = []
    for c in range(NCORES):
        sl = slice(c * HC, (c + 1) * HC)
        in_maps.append({
            "xt": xt,
            "wq": _pack_w(Wq[:, sl]),
            "wk": _pack_w(Wk[:, sl]),
            "wv": _pack_w(Wv[:, sl]),
            "wo": np.ascontiguousarray(Wo[sl, :]).astype(bf),
        })
    return in_maps


def kernel(hidden_states, Wq, Wk, Wv, Wo, bo):
    from concourse.bass_utils import run_bass_kernel_spmd

    nc = get_nc()
    in_maps = make_in_maps(hidden_states, Wq, Wk, Wv, Wo)
    res = run_bass_kernel_spmd(nc, in_maps, list(range(NCORES)))
    _CACHE["last_result"] = res
    y = np.zeros((TOK, D), np.float32)
    for c in range(NCORES):
        y += np.asarray(res.results[c]["y"], np.float32)
    out = y.reshape(B, S, D) + np.asarray(bo, np.float32)[None, None, :]
    return out.astype(np.float32)


# revision 44
# speedup vs baseline: 1.0847x; 1.0759x over previous
"""Multi-head attention block (QKV proj + softmax attention + out proj) on 8 TRN2 cores.

Sharding: head-parallel. Each core c owns heads (2c, 2c+1) for both batch elements:
  - Wq/Wk/Wv column slice [:, c*128:(c+1)*128], Wo row slice [c*128:(c+1)*128, :]
  - Q.T/K.T in [head-dim, tok] layout for scores; V in DIRECT [tok, head-dim]
    layout (projected with lhsT=X.T so no PE transposes are needed for V).
  - scores in transposed layout S.T[k, q]; exp on ACT with the 1/sqrt(d) scale
    folded into the activation pre-scale.
  - attnV in the [q, d] layout: out[128q, 65] = P.T-slice.T @ [V | ones], so the
    PE contracts over all 128 k-partitions (full utilization: 65-cycle matmuls
    instead of 512-cycle ones) and the softmax denominator lands per-q-partition
    in column 64 -> normalization is a per-partition tensor_scalar multiply.
  - O[q, hc] normalized tiles are PE-transposed to O.T[hc, tok] for the
    out-projection, which emits a partial Y = O @ Wo_c; host sums the 8 partials.

Emission is a single software-pipelined stream over all (b, qgroup, kc) slots,
grouped into BLOCKS (4 or 8 kc each). A fill queue of future-block work (proj
groups, V tiles, epilogues, out-projections) is drained between slots; a unit
consumed by block B is always fully emitted before B's first instruction
(hardware constraint from A/B testing: same-block delivery of interleaved
units produces stale reads on silicon). scores for a block's first slot are
hoisted into the previous block when their deps are already emitted, so the
ACT engine never bubbles at block boundaries.

PSUM (8 banks): sp [128,1024]f32 x2 (4 banks) + op [128,2,4,128]f32 x1
(2 banks) + yp [128,512]f32 x2 (2 banks).
"""

import numpy as np
import ml_dtypes

B = 2
S = 2048
TOK = B * S
D = 1024
HD = 64
HC = 128  # head-cols per core: 2 heads x 64
NCORES = 8
KC = D // 128  # contraction chunks for the projections
NKT = S // 128  # k-token tiles per batch
SCALE = 0.125  # 1/sqrt(HD)
QG = 512  # query-group size per attention phase
NQG = S // QG  # phases per batch
NQT = QG // 128  # 128-query tiles per phase

_CACHE = {}


def _build_nc():
    import concourse.mybir as mybir
    import concourse.tile as tile
    from concourse import bacc
    from concourse.masks import make_identity

    f32 = mybir.dt.float32
    bf16 = mybir.dt.bfloat16
    Exp = mybir.ActivationFunctionType.Exp

    nc = bacc.Bacc("TRN2", target_bir_lowering=False, debug=False, num_devices=NCORES)
    xt_d = nc.dram_tensor("xt", [D, TOK], bf16, kind="ExternalInput")
    # weights host-packed as [128, KC*HC] (partition p row = concat over
    # chunks o of W[o*128+p, :]) so each load is 2KB-contiguous per partition
    wq_d = nc.dram_tensor("wq", [128, KC * HC], bf16, kind="ExternalInput")
    wk_d = nc.dram_tensor("wk", [128, KC * HC], bf16, kind="ExternalInput")
    wv_d = nc.dram_tensor("wv", [128, KC * HC], bf16, kind="ExternalInput")
    wo_d = nc.dram_tensor("wo", [HC, D], bf16, kind="ExternalInput")
    y_d = nc.dram_tensor("y", [TOK, D], bf16, kind="ExternalOutput")

    with tile.TileContext(nc) as tc:
        with (
            tc.tile_pool(name="consts", bufs=1) as consts,
            tc.tile_pool(name="persist", bufs=1) as persist,
            tc.tile_pool(name="xqp", bufs=4) as xqp,
            tc.tile_pool(name="ptp", bufs=3) as ptp,
            tc.tile_pool(name="miscp", bufs=2) as miscp,
            tc.tile_pool(name="ysbp", bufs=6) as ysbp,
            tc.tile_pool(name="aps", space="PSUM", bufs=1) as aps,
        ):
            # --- persistent SBUF ---
            w_sb = {}

            def load_weight(nm, d, eng):
                w = consts.tile([128, KC, HC], bf16, name=f"{nm}_sb", tag=nm)
                eng.dma_start(w[:], d.rearrange("p (o m) -> p o m", o=KC))
                w_sb[nm] = w
            load_weight("wk", wk_d, nc.sync)
            wo_sb = consts.tile([HC, D], bf16, name="wo_sb", tag="wo")
            ident = consts.tile([128, 128], bf16, name="ident", tag="ident")
            make_identity(nc, ident[:])

            qt = persist.tile([HC, TOK], bf16, name="qt", tag="qt")
            kt = persist.tile([HC, TOK], bf16, name="kt", tag="kt")
            # V direct layout per (batch, ktile, head): [tok-part, 65];
            # col 64 = ones (softmax denominator rides the PV matmul).
            vp = persist.tile([128, B, NKT, 2, 65], bf16, name="vp", tag="vp")
            ot = persist.tile([HC, TOK], bf16, name="ot", tag="ot")
            nc.gpsimd.memset(vp[:, :, :, :, 64:65], 1.0)

            xt_r = xt_d.rearrange("(o p) n -> p o n", p=128)
            xq_tiles = {}

            def get_xq(tq):
                if tq not in xq_tiles:
                    xq_tiles[tq] = xqp.tile([128, KC, 1024], bf16,
                                            name=f"xq{tq}", tag=f"xq{tq}", bufs=1)
                return xq_tiles[tq]

            def load_xq_half(tq, half, eng, pair_chunks=False):
                """Load 512 tokens (half a tq tile). pair_chunks splits into
                4 DMAs of 2 contraction-chunks so the first proj matmuls can
                chase the arriving data."""
                xq = get_xq(tq)
                c0 = tq * 1024 + half * 512
                step = 2 if pair_chunks else KC
                for kc in range(0, KC, step):
                    eng.dma_start(
                        xq[:, kc:kc + step, half * 512:(half + 1) * 512],
                        xt_r[:, kc:kc + step, c0:c0 + 512])

            def load_xq(tq, eng):
                xq = get_xq(tq)
                eng.dma_start(xq[:], xt_r[:, :, tq * 1024:(tq + 1) * 1024])

            # warm the ACT exp table off the critical path
            warm = miscp.tile([1, 64], f32, name="warm", tag="warm", bufs=1)
            nc.gpsimd.memset(warm[:], 0.0)
            nc.scalar.activation(warm[:], warm[:], Exp)

            proj_dst = {"q": qt, "k": kt}

            # units yield their approximate PE-ns cost per step (fill pacing)

            def proj_group(pname, g, copy_eng=None):
                """One [128,512] Q.T/K.T projection group; yields per matmul."""
                dst, w = proj_dst[pname], w_sb["w" + pname]
                tq, nch = g // 2, g % 2
                xq = get_xq(tq)
                ps = aps.tile([128, 512], f32, name=f"ps_{pname}{g}", tag="yp",
                              bufs=2)
                for kc in range(KC):
                    nc.tensor.matmul(ps[:], w[:, kc, :],
                                     xq[:, kc, nch * 512:(nch + 1) * 512],
                                     start=(kc == 0), stop=(kc == KC - 1))
                    yield 215
                (copy_eng or nc.vector).tensor_copy(
                    out=dst[:, g * 512:(g + 1) * 512], in_=ps[:])
                yield 10

            def vproj_unit(b, t):
                """V[tok-tile, 128hc] directly (lhsT = X.T slice): no transpose."""
                tq, off = (b * S + t * 128) // 1024, (b * S + t * 128) % 1024
                xq = get_xq(tq)
                ps = aps.tile([128, 512], f32, name=f"vps{b}{t}", tag="yp", bufs=2)
                for kc in range(KC):
                    nc.tensor.matmul(ps[:, 0:128], xq[:, kc, off:off + 128],
                                     w_sb["wv"][:, kc, :],
                                     start=(kc == 0), stop=(kc == KC - 1))
                    if kc % 2 == 1:
                        yield 110
                nc.vector.tensor_copy(
                    out=vp[:, b, t, :, 0:64],
                    in_=ps[:, 0:128].rearrange("p (h m) -> p h m", h=2))
                yield 10

            def outproj_unit(pi, tt, ysb_eng):
                """Y[tok-tile, :] for one 128-token tile."""
                b, q0, qg = phases[pi]
                t0 = b * S + q0 + tt * 128
                for odc in range(2):
                    yp = aps.tile([128, 512], f32, name="yp", tag="yp", bufs=2)
                    nc.tensor.matmul(yp[:], ot[:, t0:t0 + 128],
                                     wo_sb[:, odc * 512:(odc + 1) * 512],
                                     start=True, stop=True)
                    ysb = ysbp.tile([128, 512], bf16, name="ysb", tag="ysb")
                    ysb_eng.tensor_copy(out=ysb[:], in_=yp[:])
                    nc.sync.dma_start(
                        y_d[t0:t0 + 128, odc * 512:(odc + 1) * 512], ysb[:])
                    yield 225

            def epilogue_unit(pi, osb, rr):
                """Normalize O, transpose to ot, out-project. osb/rr were
                emitted inline at the end of the phase (kc==15)."""
                b, q0, qg = phases[pi]
                t0 = b * S + q0
                nqt = qg // 128
                otd = miscp.tile([128, NQT, 128], bf16, name="otd", tag="otd",
                                 bufs=2)
                for h in range(2):
                    for qt_i in range(nqt):
                        nc.vector.tensor_scalar_mul(
                            otd[:, qt_i, h * 64:(h + 1) * 64],
                            osb[:, h, qt_i, 0:64], rr[:, h, qt_i, :])
                    yield 10
                for qt_i in range(nqt):
                    tp = aps.tile([128, 128], bf16, name="tp", tag="yp", bufs=2)
                    nc.tensor.transpose(tp[:], otd[:, qt_i, :], ident[:])
                    nc.vector.tensor_copy(
                        out=ot[:, t0 + qt_i * 128:t0 + (qt_i + 1) * 128],
                        in_=tp[:])
                    yield 70

            # ---------------- fill machinery ----------------
            from collections import deque
            fillq = deque()  # of (key, generator)
            done_keys = set()
            cur = [None]

            def _step():
                """Advance the head unit one yield; returns PE-ns cost or
                None when the queue is dry."""
                while True:
                    if cur[0] is None:
                        if not fillq:
                            return None
                        cur[0] = fillq.popleft()
                    key, gen = cur[0]
                    c = next(gen, StopIteration)
                    if c is StopIteration:
                        done_keys.add(key)
                        cur[0] = None
                        continue
                    return c

            fill_carry = [0]  # overshoot debt carried into the next slot

            def fill(budget_ns):
                avail = budget_ns + fill_carry[0]
                while avail > 0:
                    c = _step()
                    if c is None:
                        fill_carry[0] = 0  # idle PE time is not bankable
                        return
                    avail -= c
                fill_carry[0] = max(avail, -600)

            def drain_until(*keys):
                while any(k not in done_keys for k in keys):
                    if _step() is None:
                        raise RuntimeError(f"fill queue dry, missing {keys}")

            def drain_all():
                while _step() is not None:
                    pass

            def push(key, gen, front=False):
                if front:
                    fillq.appendleft((key, gen))
                else:
                    fillq.append((key, gen))

            def promote(keys):
                """Move units with the given keys (in order) to the queue
                front so deadlines for the next block are met first."""
                want = [k for k in keys if k not in done_keys]
                if cur[0] is not None and cur[0][0] in want:
                    want.remove(cur[0][0])
                if not want:
                    return
                picked = {k: None for k in want}
                rest = deque()
                for key, gen in fillq:
                    if key in picked and picked[key] is None:
                        picked[key] = gen
                    else:
                        rest.append((key, gen))
                fillq.clear()
                for k in want:
                    if picked.get(k) is not None:
                        fillq.append((k, picked[k]))
                fillq.extend(rest)

            def run_now(key, gen):
                for _ in gen:
                    pass
                done_keys.add(key)

            # ---------------- attention stream ----------------
            # phases: (b, q0, qg_width). The final phase is only 128 queries
            # so the post-last-exp tail (normalize/transpose/outproj/DMA) is
            # short; the 384 phase absorbs the remainder.
            phases = [(0, 0, 512), (0, 512, 512), (0, 1024, 512),
                      (0, 1536, 512), (1, 0, 512), (1, 512, 512),
                      (1, 1024, 512), (1, 1536, 384), (1, 1920, 128)]
            NPH = len(phases)

            # blocks: (phase, kc0, kc1). Phase 0 starts with single-kc blocks
            # so the V tiles can stream in via fill (attnV lags one slot);
            # later phases split in half to relax fill deadlines.
            blocks = []
            for pi in range(NPH):
                bounds = (0, 1, 2, 3, 4, 8, 12, 16) if pi == 0 else (0, 8, 16)
                for i in range(len(bounds) - 1):
                    blocks.append((pi, bounds[i], bounds[i + 1]))

            def block_reqs(blk):
                """Emission deps a block's slots consume: its q/k groups, and
                (since attnV lags one slot) the V tiles for kc0-1..kc1-2 plus
                the previous phase's last V tile when the block opens a phase."""
                pi, kc0, kc1 = blk
                b, q0, qg = phases[pi]
                reqs = [("q", (b * S + q0) // 512),
                        ("q", (b * S + q0 + qg - 1) // 512)]
                for kc in range(kc0, kc1):
                    reqs.append(("k", (b * S + kc * 128) // 512))
                for kc in range(max(kc0 - 1, 0), kc1 - 1):
                    reqs.append(("v", (b, kc)))
                if kc0 == 0 and pi > 0:
                    reqs.append(("v", (phases[pi - 1][0], NKT - 1)))
                return reqs

            phase_state = {}  # pi -> dict(op=...)

            def scores(pi, kc):
                b, q0, qg = phases[pi]
                t0 = b * S + q0
                k0 = b * S + kc * 128
                sp = aps.tile([128, 2 * QG], f32, name="sp", tag="sp", bufs=2)
                for h in range(2):
                    nc.tensor.matmul(
                        sp[:, h * qg:(h + 1) * qg],
                        kt[h * 64:(h + 1) * 64, k0:k0 + 128],
                        qt[h * 64:(h + 1) * 64, t0:t0 + qg],
                        start=True, stop=True)
                return sp

            # global 1-slot software pipeline for scores across blocks
            pending_sp = [None]  # scores psum for the NEXT slot, if hoisted

            slots = [(pi, kc)
                     for (pi, kc0, kc1) in blocks for kc in range(kc0, kc1)]
            slot_block = {}
            for bi, (pi, kc0, kc1) in enumerate(blocks):
                for kc in range(kc0, kc1):
                    slot_block[(pi, kc)] = bi

            # head: ALL loads on the one sync queue, strict priority order —
            # anything on a second queue jumps ahead on the shared DMA
            # engines and delays the critical head loads. Criticals first
            # (wk already queued, tokens 0:512 of X.T chunk-paired, wq, wv),
            # then the bulk in deadline order.
            load_weight("wq", wq_d, nc.sync)
            load_xq_half(0, 0, nc.sync, pair_chunks=True)
            load_weight("wv", wv_d, nc.sync)
            load_xq_half(0, 1, nc.sync)
            load_xq_half(1, 0, nc.sync)
            load_xq_half(1, 1, nc.sync)
            nc.sync.dma_start(wo_sb[:], wo_d[:])
            load_xq(2, nc.sync)
            load_xq(3, nc.sync)

            # PE p-state warmup: back-to-back dummy transposes during the DMA
            # wait, so the first real matmuls run at full clock (the PE needs
            # ~3us of continuous busyness to ramp; any idle resets it)
            wps = aps.tile([128, 128], bf16, name="wps", tag="sp", bufs=2)
            for _ in range(40):
                nc.tensor.transpose(wps[:], ident[:], ident[:])

            # k(0) and q(0) lockstepped so both chase the arriving chunk
            # pairs; q(0)'s staging copy goes to Pool so it overlaps k(0)'s
            gk, gq = proj_group("k", 0), proj_group("q", 0, nc.gpsimd)
            for _ in range(KC + 1):
                next(gk, None)
                next(gq, None)
            done_keys.update([("k", 0), ("q", 0)])
            # first two V tiles before scores(0): the PE is otherwise idle
            # while the k/q staging copies land, and idling resets the p-state
            run_now(("v", (0, 0)), vproj_unit(0, 0))
            run_now(("v", (0, 1)), vproj_unit(0, 1))

            # fill queue: remaining production in rough need-order
            for t in range(2, 4):
                push(("v", (0, t)), vproj_unit(0, t))
            push(("k", 1), proj_group("k", 1))
            for t in range(4, 8):
                push(("v", (0, t)), vproj_unit(0, t))
            push(("k", 2), proj_group("k", 2))
            push(("k", 3), proj_group("k", 3))
            for t in range(8, 16):
                push(("v", (0, t)), vproj_unit(0, t))
            push(("q", 1), proj_group("q", 1))
            push(("q", 2), proj_group("q", 2))
            push(("q", 3), proj_group("q", 3))
            push(("k", 4), proj_group("k", 4))
            push(("k", 5), proj_group("k", 5))
            for t in range(8):
                push(("v", (1, t)), vproj_unit(1, t))
            push(("q", 4), proj_group("q", 4))
            push(("k", 6), proj_group("k", 6))
            push(("k", 7), proj_group("k", 7))
            for t in range(8, 16):
                push(("v", (1, t)), vproj_unit(1, t))
            push(("q", 5), proj_group("q", 5))
            push(("q", 6), proj_group("q", 6))
            push(("q", 7), proj_group("q", 7))

            def attnV(pi, kc, pt):
                b, q0, qg = phases[pi]
                nqt = qg // 128
                op = phase_state[pi]["op"]
                for h in range(2):
                    for qt_i in range(nqt):
                        nc.tensor.matmul(
                            op[:, h, qt_i, 0:65],
                            pt[:, h * qg + qt_i * 128:h * qg + (qt_i + 1) * 128],
                            vp[:, b, kc, h, 0:65],
                            start=(kc == 0), stop=(kc == NKT - 1))
                if kc == NKT - 1:
                    if pi == NPH - 1:
                        final_tail(pi, op)
                        del phase_state[pi]
                        return
                    # read psum out NOW (frees op for the next phase), inline
                    osb = miscp.tile([128, 2, NQT, 65], f32, name="osb",
                                     tag="osb", bufs=2)
                    nc.vector.tensor_copy(out=osb[:, :, 0:nqt, :],
                                          in_=op[:, :, 0:nqt, 0:65])
                    rr = miscp.tile([128, 2, NQT, 1], f32, name="rr", tag="rr",
                                    bufs=2)
                    nc.vector.reciprocal(rr[:, :, 0:nqt, :],
                                         osb[:, :, 0:nqt, 64:65])
                    for tt in range(nqt):
                        eng = nc.vector if tt % 2 == 0 else nc.gpsimd
                        push(("op", (pi, tt)), outproj_unit(pi, tt, eng))
                    push(("epi", pi), epilogue_unit(pi, osb, rr), front=True)
                    del phase_state[pi]

            def final_tail(pi, op):
                """Last (small) phase: pipelined normalize/transpose/outproj
                finish emitted inline after the last attnV."""
                b, q0, qg = phases[pi]
                t0 = b * S + q0
                nqt = qg // 128
                osb = miscp.tile([128, 2, NQT, 65], f32, name="osbF",
                                 tag="osb", bufs=2)
                rr = miscp.tile([128, 2, NQT, 1], f32, name="rrF", tag="rr",
                                bufs=2)
                otd = miscp.tile([128, NQT, 128], bf16, name="otdF", tag="otd",
                                 bufs=2)
                for qt_i in range(nqt):
                    nc.vector.tensor_copy(out=osb[:, :, qt_i, :],
                                          in_=op[:, :, qt_i, 0:65])
                    nc.vector.reciprocal(rr[:, :, qt_i, :],
                                         osb[:, :, qt_i, 64:65])
                    for h in range(2):
                        nc.vector.tensor_scalar_mul(
                            otd[:, qt_i, h * 64:(h + 1) * 64],
                            osb[:, h, qt_i, 0:64], rr[:, h, qt_i, :])
                    tp = aps.tile([128, 128], bf16, name="tpF", tag="yp",
                                  bufs=2)
                    nc.tensor.transpose(tp[:], otd[:, qt_i, :], ident[:])
                    nc.vector.tensor_copy(
                        out=ot[:, t0 + qt_i * 128:t0 + (qt_i + 1) * 128],
                        in_=tp[:])
                    tq0 = t0 + qt_i * 128
                    for odc in range(2):
                        yp = aps.tile([128, 512], f32, name="ypF", tag="yp",
                                      bufs=2)
                        nc.tensor.matmul(yp[:], ot[:, tq0:tq0 + 128],
                                         wo_sb[:, odc * 512:(odc + 1) * 512],
                                         start=True, stop=True)
                        ysb = ysbp.tile([128, 512], bf16, name="ysbF",
                                        tag="ysb")
                        eng = nc.vector if odc == 0 else nc.gpsimd
                        eng.tensor_copy(out=ysb[:], in_=yp[:])
                        nc.sync.dma_start(
                            y_d[tq0:tq0 + 128, odc * 512:(odc + 1) * 512],
                            ysb[:])

            def sc_reqs(npi, nkc):
                nb, nq0, nqg = phases[npi]
                return (("q", (nb * S + nq0) // 512),
                        ("q", (nb * S + nq0 + nqg - 1) // 512),
                        ("k", (nb * S + nkc * 128) // 512))

            lagged = [None]  # (pi, kc, pt) whose attnV is one slot behind

            for si, (pi, kc) in enumerate(slots):
                b, q0, qg = phases[pi]
                bi = slot_block[(pi, kc)]
                blk = blocks[bi]
                if kc == blk[1]:  # block start
                    drain_until(*block_reqs(blk))
                    if bi + 1 < len(blocks):
                        promote(block_reqs(blocks[bi + 1]))
                if kc == 0:
                    phase_state[pi] = {
                        "op": aps.tile([128, 2, NQT, 128], f32,
                                       name=f"op{pi}", tag="op", bufs=1)}
                # scores for this slot: hoisted already, or emit now
                if pending_sp[0] is not None:
                    sp_cur = pending_sp[0]
                    pending_sp[0] = None
                else:
                    sp_cur = scores(pi, kc)

                # hoist scores for the next slot if its own deps (its q-group
                # and k-group, not the whole block's) are emitted
                nxt = slots[si + 1] if si + 1 < len(slots) else None
                if nxt is not None:
                    if slot_block[nxt] == bi or all(
                            k in done_keys for k in sc_reqs(*nxt)):
                        pending_sp[0] = scores(*nxt)

                pt = ptp.tile([128, 2 * QG], bf16, name="pt", tag="pt", bufs=3)
                nc.scalar.activation(pt[:, 0:2 * qg], sp_cur[:, 0:2 * qg],
                                     Exp, scale=SCALE)

                # fill runs in the window where attnV would otherwise make
                # the in-order PE queue wait on exp: budget = ACT slot time
                # minus this slot's own PE pipeline work minus margin
                act_ns = int(2 * qg * 0.833) + 185
                pipe_ns = 2 * int(qg * 0.42) + (qg // 128) * 54
                fill(act_ns - pipe_ns - 40)

                # second chance: deps may have completed during this slot's
                # fill (cross-block hoist, boundary slots only)
                if nxt is not None and pending_sp[0] is None and all(
                        k in done_keys for k in sc_reqs(*nxt)):
                    pending_sp[0] = scores(*nxt)

                # attnV one slot behind: its exp finished during the previous
                # slot, so the PE never stalls on it
                if lagged[0] is not None:
                    attnV(*lagged[0])
                lagged[0] = (pi, kc, pt)
            # flush leftovers first so nothing is emitted after (and thus
            # executes after) the final per-qt tail
            drain_all()
            attnV(*lagged[0])
    nc.compile()
    return nc


def get_nc():
    if "nc" not in _CACHE:
        _CACHE["nc"] = _build_nc()
    return _CACHE["nc"]


def _pack_w(w_slice):
    """[D, HC] -> [128, KC*HC]: partition p row = concat_o W[o*128+p, :]."""
    bf = ml_dtypes.bfloat16
    return np.ascontiguousarray(
        w_slice.reshape(KC, 128, HC).transpose(1, 0, 2).reshape(128, KC * HC)
    ).astype(bf)


def make_in_maps(hidden_states, Wq, Wk, Wv, Wo):
    bf = ml_dtypes.bfloat16
    X = np.ascontiguousarray(np.asarray(hidden_states, np.float32).reshape(TOK, D))
    xt = np.ascontiguousarray(X.T).astype(bf)
    Wq = np.asarray(Wq, np.float32)
    Wk = np.asarray(Wk, np.float32)
    Wv = np.asarray(Wv, np.float32)
    Wo = np.asarray(Wo, np.float32)
    in_maps = []
    for c in range(NCORES):
        sl = slice(c * HC, (c + 1) * HC)
        in_maps.append({
            "xt": xt,
            "wq": _pack_w(Wq[:, sl]),
            "wk": _pack_w(Wk[:, sl]),
            "wv": _pack_w(Wv[:, sl]),
            "wo": np.ascontiguousarray(Wo[sl, :]).astype(bf),
        })
    return in_maps


def kernel(hidden_states, Wq, Wk, Wv, Wo, bo):
    from concourse.bass_utils import run_bass_kernel_spmd

    nc = get_nc()
    in_maps = make_in_maps(hidden_states, Wq, Wk, Wv, Wo)
    res = run_bass_kernel_spmd(nc, in_maps, list(range(NCORES)))
    _CACHE["last_result"] = res
    y = np.zeros((TOK, D), np.float32)
    for c in range(NCORES):
        y += np.asarray(res.results[c]["y"], np.float32)
    out = y.reshape(B, S, D) + np.asarray(bo, np.float32)[None, None, :]
    return out.astype(np.float32)


# revision 45
# speedup vs baseline: 1.0902x; 1.0050x over previous
"""Multi-head attention block (QKV proj + softmax attention + out proj) on 8 TRN2 cores.

Sharding: head-parallel. Each core c owns heads (2c, 2c+1) for both batch elements:
  - Wq/Wk/Wv column slice [:, c*128:(c+1)*128], Wo row slice [c*128:(c+1)*128, :]
  - Q.T/K.T in [head-dim, tok] layout for scores; V in DIRECT [tok, head-dim]
    layout (projected with lhsT=X.T so no PE transposes are needed for V).
  - scores in transposed layout S.T[k, q]; exp on ACT with the 1/sqrt(d) scale
    folded into the activation pre-scale.
  - attnV in the [q, d] layout: out[128q, 65] = P.T-slice.T @ [V | ones], so the
    PE contracts over all 128 k-partitions (full utilization: 65-cycle matmuls
    instead of 512-cycle ones) and the softmax denominator lands per-q-partition
    in column 64 -> normalization is a per-partition tensor_scalar multiply.
  - O[q, hc] normalized tiles are PE-transposed to O.T[hc, tok] for the
    out-projection, which emits a partial Y = O @ Wo_c; host sums the 8 partials.

Emission is a single software-pipelined stream over all (b, qgroup, kc) slots,
grouped into BLOCKS (4 or 8 kc each). A fill queue of future-block work (proj
groups, V tiles, epilogues, out-projections) is drained between slots; a unit
consumed by block B is always fully emitted before B's first instruction
(hardware constraint from A/B testing: same-block delivery of interleaved
units produces stale reads on silicon). scores for a block's first slot are
hoisted into the previous block when their deps are already emitted, so the
ACT engine never bubbles at block boundaries.

PSUM (8 banks): sp [128,1024]f32 x2 (4 banks) + op [128,2,4,128]f32 x1
(2 banks) + yp [128,512]f32 x2 (2 banks).
"""

import numpy as np
import ml_dtypes

B = 2
S = 2048
TOK = B * S
D = 1024
HD = 64
HC = 128  # head-cols per core: 2 heads x 64
NCORES = 8
KC = D // 128  # contraction chunks for the projections
NKT = S // 128  # k-token tiles per batch
SCALE = 0.125  # 1/sqrt(HD)
QG = 512  # query-group size per attention phase
NQG = S // QG  # phases per batch
NQT = QG // 128  # 128-query tiles per phase

_CACHE = {}


def _build_nc():
    import concourse.mybir as mybir
    import concourse.tile as tile
    from concourse import bacc
    from concourse.masks import make_identity

    f32 = mybir.dt.float32
    bf16 = mybir.dt.bfloat16
    Exp = mybir.ActivationFunctionType.Exp

    nc = bacc.Bacc("TRN2", target_bir_lowering=False, debug=False, num_devices=NCORES)
    xt_d = nc.dram_tensor("xt", [D, TOK], bf16, kind="ExternalInput")
    # weights host-packed as [128, KC*HC] (partition p row = concat over
    # chunks o of W[o*128+p, :]) so each load is 2KB-contiguous per partition
    wq_d = nc.dram_tensor("wq", [128, KC * HC], bf16, kind="ExternalInput")
    wk_d = nc.dram_tensor("wk", [128, KC * HC], bf16, kind="ExternalInput")
    wv_d = nc.dram_tensor("wv", [128, KC * HC], bf16, kind="ExternalInput")
    wo_d = nc.dram_tensor("wo", [HC, D], bf16, kind="ExternalInput")
    y_d = nc.dram_tensor("y", [TOK, D], bf16, kind="ExternalOutput")

    with tile.TileContext(nc) as tc:
        with (
            tc.tile_pool(name="consts", bufs=1) as consts,
            tc.tile_pool(name="persist", bufs=1) as persist,
            tc.tile_pool(name="xqp", bufs=4) as xqp,
            tc.tile_pool(name="ptp", bufs=3) as ptp,
            tc.tile_pool(name="miscp", bufs=2) as miscp,
            tc.tile_pool(name="ysbp", bufs=6) as ysbp,
            tc.tile_pool(name="aps", space="PSUM", bufs=1) as aps,
        ):
            # --- persistent SBUF ---
            w_sb = {}

            def load_weight(nm, d, eng):
                w = consts.tile([128, KC, HC], bf16, name=f"{nm}_sb", tag=nm)
                eng.dma_start(w[:], d.rearrange("p (o m) -> p o m", o=KC))
                w_sb[nm] = w
            load_weight("wk", wk_d, nc.sync)
            wo_sb = consts.tile([HC, D], bf16, name="wo_sb", tag="wo")
            ident = consts.tile([128, 128], bf16, name="ident", tag="ident")
            make_identity(nc, ident[:])

            qt = persist.tile([HC, TOK], bf16, name="qt", tag="qt")
            kt = persist.tile([HC, TOK], bf16, name="kt", tag="kt")
            # V direct layout per (batch, ktile, head): [tok-part, 65];
            # col 64 = ones (softmax denominator rides the PV matmul).
            vp = persist.tile([128, B, NKT, 2, 65], bf16, name="vp", tag="vp")
            ot = persist.tile([HC, TOK], bf16, name="ot", tag="ot")
            nc.gpsimd.memset(vp[:, :, :, :, 64:65], 1.0)

            xt_r = xt_d.rearrange("(o p) n -> p o n", p=128)
            xq_tiles = {}

            def get_xq(tq):
                if tq not in xq_tiles:
                    xq_tiles[tq] = xqp.tile([128, KC, 1024], bf16,
                                            name=f"xq{tq}", tag=f"xq{tq}", bufs=1)
                return xq_tiles[tq]

            def load_xq_half(tq, half, eng, pair_chunks=False):
                """Load 512 tokens (half a tq tile). pair_chunks splits into
                4 DMAs of 2 contraction-chunks so the first proj matmuls can
                chase the arriving data."""
                xq = get_xq(tq)
                c0 = tq * 1024 + half * 512
                step = 2 if pair_chunks else KC
                for kc in range(0, KC, step):
                    eng.dma_start(
                        xq[:, kc:kc + step, half * 512:(half + 1) * 512],
                        xt_r[:, kc:kc + step, c0:c0 + 512])

            def load_xq(tq, eng):
                xq = get_xq(tq)
                eng.dma_start(xq[:], xt_r[:, :, tq * 1024:(tq + 1) * 1024])

            # warm the ACT exp table off the critical path
            warm = miscp.tile([1, 64], f32, name="warm", tag="warm", bufs=1)
            nc.gpsimd.memset(warm[:], 0.0)
            nc.scalar.activation(warm[:], warm[:], Exp)

            proj_dst = {"q": qt, "k": kt}

            # units yield their approximate PE-ns cost per step (fill pacing)

            def proj_group(pname, g, copy_eng=None):
                """One [128,512] Q.T/K.T projection group; yields per matmul."""
                dst, w = proj_dst[pname], w_sb["w" + pname]
                tq, nch = g // 2, g % 2
                xq = get_xq(tq)
                ps = aps.tile([128, 512], f32, name=f"ps_{pname}{g}", tag="yp",
                              bufs=2)
                for kc in range(KC):
                    nc.tensor.matmul(ps[:], w[:, kc, :],
                                     xq[:, kc, nch * 512:(nch + 1) * 512],
                                     start=(kc == 0), stop=(kc == KC - 1))
                    yield 215
                (copy_eng or nc.vector).tensor_copy(
                    out=dst[:, g * 512:(g + 1) * 512], in_=ps[:])
                yield 10

            def vproj_unit(b, t):
                """V[tok-tile, 128hc] directly (lhsT = X.T slice): no transpose."""
                tq, off = (b * S + t * 128) // 1024, (b * S + t * 128) % 1024
                xq = get_xq(tq)
                ps = aps.tile([128, 512], f32, name=f"vps{b}{t}", tag="yp", bufs=2)
                for kc in range(KC):
                    nc.tensor.matmul(ps[:, 0:128], xq[:, kc, off:off + 128],
                                     w_sb["wv"][:, kc, :],
                                     start=(kc == 0), stop=(kc == KC - 1))
                    if kc % 2 == 1:
                        yield 110
                nc.vector.tensor_copy(
                    out=vp[:, b, t, :, 0:64],
                    in_=ps[:, 0:128].rearrange("p (h m) -> p h m", h=2))
                yield 10

            def outproj_unit(pi, tt, ysb_eng):
                """Y[tok-tile, :] for one 128-token tile."""
                b, q0, qg = phases[pi]
                t0 = b * S + q0 + tt * 128
                for odc in range(2):
                    yp = aps.tile([128, 512], f32, name="yp", tag="yp", bufs=2)
                    nc.tensor.matmul(yp[:], ot[:, t0:t0 + 128],
                                     wo_sb[:, odc * 512:(odc + 1) * 512],
                                     start=True, stop=True)
                    ysb = ysbp.tile([128, 512], bf16, name="ysb", tag="ysb")
                    ysb_eng.tensor_copy(out=ysb[:], in_=yp[:])
                    nc.sync.dma_start(
                        y_d[t0:t0 + 128, odc * 512:(odc + 1) * 512], ysb[:])
                    yield 225

            def epilogue_unit(pi, osb, rr):
                """Normalize O, transpose to ot, out-project. osb/rr were
                emitted inline at the end of the phase (kc==15)."""
                b, q0, qg = phases[pi]
                t0 = b * S + q0
                nqt = qg // 128
                otd = miscp.tile([128, NQT, 128], bf16, name="otd", tag="otd",
                                 bufs=2)
                for h in range(2):
                    for qt_i in range(nqt):
                        nc.vector.tensor_scalar_mul(
                            otd[:, qt_i, h * 64:(h + 1) * 64],
                            osb[:, h, qt_i, 0:64], rr[:, h, qt_i, :])
                    yield 10
                for qt_i in range(nqt):
                    tp = aps.tile([128, 128], bf16, name="tp", tag="yp", bufs=2)
                    nc.tensor.transpose(tp[:], otd[:, qt_i, :], ident[:])
                    nc.vector.tensor_copy(
                        out=ot[:, t0 + qt_i * 128:t0 + (qt_i + 1) * 128],
                        in_=tp[:])
                    yield 70

            # ---------------- fill machinery ----------------
            from collections import deque
            fillq = deque()  # of (key, generator)
            done_keys = set()
            cur = [None]

            def _step():
                """Advance the head unit one yield; returns PE-ns cost or
                None when the queue is dry."""
                while True:
                    if cur[0] is None:
                        if not fillq:
                            return None
                        cur[0] = fillq.popleft()
                    key, gen = cur[0]
                    c = next(gen, StopIteration)
                    if c is StopIteration:
                        done_keys.add(key)
                        cur[0] = None
                        continue
                    return c

            fill_carry = [0]  # overshoot debt carried into the next slot

            def fill(budget_ns):
                avail = budget_ns + fill_carry[0]
                while avail > 0:
                    c = _step()
                    if c is None:
                        fill_carry[0] = 0  # idle PE time is not bankable
                        return
                    avail -= c
                fill_carry[0] = max(avail, -600)

            def drain_until(*keys):
                while any(k not in done_keys for k in keys):
                    if _step() is None:
                        raise RuntimeError(f"fill queue dry, missing {keys}")

            def drain_all():
                while _step() is not None:
                    pass

            def push(key, gen, front=False):
                if front:
                    fillq.appendleft((key, gen))
                else:
                    fillq.append((key, gen))

            def promote(keys):
                """Move units with the given keys (in order) to the queue
                front so deadlines for the next block are met first."""
                want = [k for k in keys if k not in done_keys]
                if cur[0] is not None and cur[0][0] in want:
                    want.remove(cur[0][0])
                if not want:
                    return
                picked = {k: None for k in want}
                rest = deque()
                for key, gen in fillq:
                    if key in picked and picked[key] is None:
                        picked[key] = gen
                    else:
                        rest.append((key, gen))
                fillq.clear()
                for k in want:
                    if picked.get(k) is not None:
                        fillq.append((k, picked[k]))
                fillq.extend(rest)

            def run_now(key, gen):
                for _ in gen:
                    pass
                done_keys.add(key)

            # ---------------- attention stream ----------------
            # phases: (b, q0, qg_width). The final phase is only 128 queries
            # so the post-last-exp tail (normalize/transpose/outproj/DMA) is
            # short; the 384 phase absorbs the remainder.
            phases = [(0, 0, 512), (0, 512, 512), (0, 1024, 512),
                      (0, 1536, 512), (1, 0, 512), (1, 512, 512),
                      (1, 1024, 512), (1, 1536, 384), (1, 1920, 128)]
            NPH = len(phases)

            # blocks: (phase, kc0, kc1). Phase 0 starts with single-kc blocks
            # so the V tiles can stream in via fill (attnV lags one slot);
            # later phases split in half to relax fill deadlines.
            blocks = []
            for pi in range(NPH):
                bounds = (0, 1, 2, 3, 4, 8, 12, 16) if pi == 0 else (0, 8, 16)
                for i in range(len(bounds) - 1):
                    blocks.append((pi, bounds[i], bounds[i + 1]))

            def block_reqs(blk):
                """Emission deps a block's slots consume: its q/k groups, and
                (since attnV lags one slot) the V tiles for kc0-1..kc1-2 plus
                the previous phase's last V tile when the block opens a phase."""
                pi, kc0, kc1 = blk
                b, q0, qg = phases[pi]
                reqs = [("q", (b * S + q0) // 512),
                        ("q", (b * S + q0 + qg - 1) // 512)]
                for kc in range(kc0, kc1):
                    reqs.append(("k", (b * S + kc * 128) // 512))
                for kc in range(max(kc0 - 1, 0), kc1 - 1):
                    reqs.append(("v", (b, kc)))
                if kc0 == 0 and pi > 0:
                    reqs.append(("v", (phases[pi - 1][0], NKT - 1)))
                return reqs

            phase_state = {}  # pi -> dict(op=...)

            def scores(pi, kc):
                b, q0, qg = phases[pi]
                t0 = b * S + q0
                k0 = b * S + kc * 128
                sp = aps.tile([128, 2 * QG], f32, name="sp", tag="sp", bufs=2)
                for h in range(2):
                    nc.tensor.matmul(
                        sp[:, h * qg:(h + 1) * qg],
                        kt[h * 64:(h + 1) * 64, k0:k0 + 128],
                        qt[h * 64:(h + 1) * 64, t0:t0 + qg],
                        start=True, stop=True)
                return sp

            # global 1-slot software pipeline for scores across blocks
            pending_sp = [None]  # scores psum for the NEXT slot, if hoisted

            slots = [(pi, kc)
                     for (pi, kc0, kc1) in blocks for kc in range(kc0, kc1)]
            slot_block = {}
            for bi, (pi, kc0, kc1) in enumerate(blocks):
                for kc in range(kc0, kc1):
                    slot_block[(pi, kc)] = bi

            # head: ALL loads on the one sync queue, strict priority order —
            # anything on a second queue jumps ahead on the shared DMA
            # engines and delays the critical head loads. Criticals first
            # (wk already queued, tokens 0:512 of X.T chunk-paired, wq, wv),
            # then the bulk in deadline order.
            load_weight("wq", wq_d, nc.sync)
            load_xq_half(0, 0, nc.sync, pair_chunks=True)
            load_weight("wv", wv_d, nc.sync)
            load_xq_half(0, 1, nc.sync)
            load_xq_half(1, 0, nc.sync)
            load_xq_half(1, 1, nc.sync)
            nc.sync.dma_start(wo_sb[:], wo_d[:])
            load_xq(2, nc.sync)
            load_xq(3, nc.sync)

            # PE p-state warmup: back-to-back dummy transposes during the DMA
            # wait, so the first real matmuls run at full clock (the PE needs
            # ~3us of continuous busyness to ramp; any idle resets it)
            wps = aps.tile([128, 128], bf16, name="wps", tag="sp", bufs=2)
            for _ in range(40):
                nc.tensor.transpose(wps[:], ident[:], ident[:])

            # k(0) and q(0) lockstepped so both chase the arriving chunk
            # pairs; q(0)'s staging copy goes to Pool so it overlaps k(0)'s
            gk, gq = proj_group("k", 0), proj_group("q", 0)
            for _ in range(KC + 1):
                next(gk, None)
                next(gq, None)
            done_keys.update([("k", 0), ("q", 0)])
            # first two V tiles before scores(0): the PE is otherwise idle
            # while the k/q staging copies land, and idling resets the p-state
            run_now(("v", (0, 0)), vproj_unit(0, 0))
            run_now(("v", (0, 1)), vproj_unit(0, 1))

            # fill queue: remaining production in rough need-order
            for t in range(2, 4):
                push(("v", (0, t)), vproj_unit(0, t))
            push(("k", 1), proj_group("k", 1))
            for t in range(4, 8):
                push(("v", (0, t)), vproj_unit(0, t))
            push(("k", 2), proj_group("k", 2))
            push(("k", 3), proj_group("k", 3))
            for t in range(8, 16):
                push(("v", (0, t)), vproj_unit(0, t))
            push(("q", 1), proj_group("q", 1))
            push(("q", 2), proj_group("q", 2))
            push(("q", 3), proj_group("q", 3))
            push(("k", 4), proj_group("k", 4))
            push(("k", 5), proj_group("k", 5))
            for t in range(8):
                push(("v", (1, t)), vproj_unit(1, t))
            push(("q", 4), proj_group("q", 4))
            push(("k", 6), proj_group("k", 6))
            push(("k", 7), proj_group("k", 7))
            for t in range(8, 16):
                push(("v", (1, t)), vproj_unit(1, t))
            push(("q", 5), proj_group("q", 5))
            push(("q", 6), proj_group("q", 6))
            push(("q", 7), proj_group("q", 7))

            def attnV(pi, kc, pt):
                b, q0, qg = phases[pi]
                nqt = qg // 128
                op = phase_state[pi]["op"]
                for h in range(2):
                    for qt_i in range(nqt):
                        nc.tensor.matmul(
                            op[:, h, qt_i, 0:65],
                            pt[:, h * qg + qt_i * 128:h * qg + (qt_i + 1) * 128],
                            vp[:, b, kc, h, 0:65],
                            start=(kc == 0), stop=(kc == NKT - 1))
                if kc == NKT - 1:
                    if pi == NPH - 1:
                        final_tail(pi, op)
                        del phase_state[pi]
                        return
                    # read psum out NOW (frees op for the next phase), inline
                    osb = miscp.tile([128, 2, NQT, 65], f32, name="osb",
                                     tag="osb", bufs=2)
                    nc.vector.tensor_copy(out=osb[:, :, 0:nqt, :],
                                          in_=op[:, :, 0:nqt, 0:65])
                    rr = miscp.tile([128, 2, NQT, 1], f32, name="rr", tag="rr",
                                    bufs=2)
                    nc.vector.reciprocal(rr[:, :, 0:nqt, :],
                                         osb[:, :, 0:nqt, 64:65])
                    for tt in range(nqt):
                        push(("op", (pi, tt)), outproj_unit(pi, tt, nc.vector))
                    push(("epi", pi), epilogue_unit(pi, osb, rr), front=True)
                    del phase_state[pi]

            def final_tail(pi, op):
                """Last (small) phase: pipelined normalize/transpose/outproj
                finish emitted inline after the last attnV."""
                b, q0, qg = phases[pi]
                t0 = b * S + q0
                nqt = qg // 128
                osb = miscp.tile([128, 2, NQT, 65], f32, name="osbF",
                                 tag="osb", bufs=2)
                rr = miscp.tile([128, 2, NQT, 1], f32, name="rrF", tag="rr",
                                bufs=2)
                otd = miscp.tile([128, NQT, 128], bf16, name="otdF", tag="otd",
                                 bufs=2)
                for qt_i in range(nqt):
                    nc.vector.tensor_copy(out=osb[:, :, qt_i, :],
                                          in_=op[:, :, qt_i, 0:65])
                    nc.vector.reciprocal(rr[:, :, qt_i, :],
                                         osb[:, :, qt_i, 64:65])
                    for h in range(2):
                        nc.vector.tensor_scalar_mul(
                            otd[:, qt_i, h * 64:(h + 1) * 64],
                            osb[:, h, qt_i, 0:64], rr[:, h, qt_i, :])
                    tp = aps.tile([128, 128], bf16, name="tpF", tag="yp",
                                  bufs=2)
                    nc.tensor.transpose(tp[:], otd[:, qt_i, :], ident[:])
                    nc.vector.tensor_copy(
                        out=ot[:, t0 + qt_i * 128:t0 + (qt_i + 1) * 128],
                        in_=tp[:])
                    tq0 = t0 + qt_i * 128
                    for odc in range(2):
                        yp = aps.tile([128, 512], f32, name="ypF", tag="yp",
                                      bufs=2)
                        nc.tensor.matmul(yp[:], ot[:, tq0:tq0 + 128],
                                         wo_sb[:, odc * 512:(odc + 1) * 512],
                                         start=True, stop=True)
                        ysb = ysbp.tile([128, 512], bf16, name="ysbF",
                                        tag="ysb")
                        if odc == 0:
                            nc.vector.tensor_copy(out=ysb[:], in_=yp[:])
                        else:
                            # ACT is idle after the last exp; GPSIMD cannot
                            # read PSUM on real hardware
                            nc.scalar.copy(out=ysb[:], in_=yp[:])
                        nc.sync.dma_start(
                            y_d[tq0:tq0 + 128, odc * 512:(odc + 1) * 512],
                            ysb[:])

            def sc_reqs(npi, nkc):
                nb, nq0, nqg = phases[npi]
                return (("q", (nb * S + nq0) // 512),
                        ("q", (nb * S + nq0 + nqg - 1) // 512),
                        ("k", (nb * S + nkc * 128) // 512))

            lagged = [None]  # (pi, kc, pt) whose attnV is one slot behind

            for si, (pi, kc) in enumerate(slots):
                b, q0, qg = phases[pi]
                bi = slot_block[(pi, kc)]
                blk = blocks[bi]
                if kc == blk[1]:  # block start
                    drain_until(*block_reqs(blk))
                    if bi + 1 < len(blocks):
                        promote(block_reqs(blocks[bi + 1]))
                if kc == 0:
                    phase_state[pi] = {
                        "op": aps.tile([128, 2, NQT, 128], f32,
                                       name=f"op{pi}", tag="op", bufs=1)}
                # scores for this slot: hoisted already, or emit now
                if pending_sp[0] is not None:
                    sp_cur = pending_sp[0]
                    pending_sp[0] = None
                else:
                    sp_cur = scores(pi, kc)

                # hoist scores for the next slot if its own deps (its q-group
                # and k-group, not the whole block's) are emitted
                nxt = slots[si + 1] if si + 1 < len(slots) else None
                if nxt is not None:
                    if slot_block[nxt] == bi or all(
                            k in done_keys for k in sc_reqs(*nxt)):
                        pending_sp[0] = scores(*nxt)

                pt = ptp.tile([128, 2 * QG], bf16, name="pt", tag="pt", bufs=3)
                nc.scalar.activation(pt[:, 0:2 * qg], sp_cur[:, 0:2 * qg],
                                     Exp, scale=SCALE)

                # fill runs in the window where attnV would otherwise make
                # the in-order PE queue wait on exp: budget = ACT slot time
                # minus this slot's own PE pipeline work minus margin
                act_ns = int(2 * qg * 0.833) + 185
                pipe_ns = 2 * int(qg * 0.42) + (qg // 128) * 54
                fill(act_ns - pipe_ns - 40)

                # second chance: deps may have completed during this slot's
                # fill (cross-block hoist, boundary slots only)
                if nxt is not None and pending_sp[0] is None and all(
                        k in done_keys for k in sc_reqs(*nxt)):
                    pending_sp[0] = scores(*nxt)

                # attnV one slot behind: its exp finished during the previous
                # slot, so the PE never stalls on it
                if lagged[0] is not None:
                    attnV(*lagged[0])
                lagged[0] = (pi, kc, pt)
            # flush leftovers first so nothing is emitted after (and thus
            # executes after) the final per-qt tail
            drain_all()
            attnV(*lagged[0])
    nc.compile()
    return nc


def get_nc():
    if "nc" not in _CACHE:
        _CACHE["nc"] = _build_nc()
    return _CACHE["nc"]


def _pack_w(w_slice):
    """[D, HC] -> [128, KC*HC]: partition p row = concat_o W[o*128+p, :]."""
    bf = ml_dtypes.bfloat16
    return np.ascontiguousarray(
        w_slice.reshape(KC, 128, HC).transpose(1, 0, 2).reshape(128, KC * HC)
    ).astype(bf)


def make_in_maps(hidden_states, Wq, Wk, Wv, Wo):
    bf = ml_dtypes.bfloat16
    X = np.ascontiguousarray(np.asarray(hidden_states, np.float32).reshape(TOK, D))
    xt = np.ascontiguousarray(X.T).astype(bf)
    Wq = np.asarray(Wq, np.float32)
    Wk = np.asarray(Wk, np.float32)
    Wv = np.asarray(Wv, np.float32)
    Wo = np.asarray(Wo, np.float32)
    in_maps = []
    for c in range(NCORES):
        sl = slice(c * HC, (c + 1) * HC)
        in_maps.append({
            "xt": xt,
            "wq": _pack_w(Wq[:, sl]),
            "wk": _pack_w(Wk[:, sl]),
            "wv": _pack_w(Wv[:, sl]),
            "wo": np.ascontiguousarray(Wo[sl, :]).astype(bf),
        })
    return in_maps


def kernel(hidden_states, Wq, Wk, Wv, Wo, bo):
    from concourse.bass_utils import run_bass_kernel_spmd

    nc = get_nc()
    in_maps = make_in_maps(hidden_states, Wq, Wk, Wv, Wo)
    res = run_bass_kernel_spmd(nc, in_maps, list(range(NCORES)))
    _CACHE["last_result"] = res
    y = np.zeros((TOK, D), np.float32)
    for c in range(NCORES):
        y += np.asarray(res.results[c]["y"], np.float32)
    out = y.reshape(B, S, D) + np.asarray(bo, np.float32)[None, None, :]
    return out.astype(np.float32)
